# revision 1
# baseline (speedup 1.0000x reference)
"""Trainium2 Bass kernel for nn_ConsciousnessMonitor (histogram_binning).

kernel(**inputs) takes FULL unsharded numpy inputs, returns the full (9,)
float32 output. Shards state_history along time across 8 NeuronCores:
masked means via PE matmul while streaming, min/max + joint-histogram MI
with two small AllReduces, differentiation branch replicated per core.

Self-contained: shapes/sharding hardcoded; reads no sibling files.
"""
import numpy as np

import concourse.bacc as bacc
import concourse.tile as tile
import concourse.mybir as mybir
from concourse.bass_utils import run_bass_kernel_spmd
from concourse.masks import make_identity

F32 = mybir.dt.float32
I32 = mybir.dt.int32
AX = mybir.AxisListType
OP = mybir.AluOpType
ACT = mybir.ActivationFunctionType

N_CORES = 8
T, D = 32768, 2048
TL = T // N_CORES          # 4096 time steps per core
NB = 10                    # histogram bins per axis
NPAIR = 4                  # partitions (mask pairs)
J = 2 * NPAIR              # 8 masked-mean columns
NTC = TL // 512            # 8 accumulator groups (512 t each)
NDC = D // 128             # 16 contraction chunks
NCH = TL // 128            # 32 binning chunks of 128 t
MEM = 100
SN = 10

# accumulator tcn -> (bank b, quadrant q): tcn = 3*b + q, q in {0,1,2}
ACC_MAP = [(tcn // 3, tcn % 3) for tcn in range(NTC)]

_CACHE = {}
LAST_RESULTS = None


def _build(debug=False, variant="main"):
    sim1 = variant.startswith("sim1")
    nc = bacc.Bacc("TRN2", target_bir_lowering=False, debug=False,
                   num_devices=1 if sim1 else N_CORES)
    ht = nc.dram_tensor("ht", [D, TL], F32, kind="ExternalInput").ap()
    mmat = nc.dram_tensor("mmat", [D, J], F32, kind="ExternalInput").ap()
    invc = nc.dram_tensor("invc", [128, 1], F32, kind="ExternalInput").ap()
    memt = nc.dram_tensor("memt", [D, MEM], F32, kind="ExternalInput").ap()
    sampt = nc.dram_tensor("sampt", [D, SN], F32, kind="ExternalInput").ap()
    out = nc.dram_tensor("out", [9], F32, kind="ExternalOutput").ap()
    if debug:
        dbg_st = nc.dram_tensor("dbg_st", [J, 512], F32, kind="ExternalOutput").ap()
        dbg_gmm = nc.dram_tensor("dbg_gmm", [J, 2], F32, kind="ExternalOutput").ap()
        dbg_rmat = nc.dram_tensor("dbg_rmat", [J + 1, J], F32, kind="ExternalOutput").ap()
        dbg_bin = nc.dram_tensor("dbg_bin", [128, 16], I32, kind="ExternalOutput").ap()
        dbg_gj = nc.dram_tensor("dbg_gj", [NB, NPAIR * NB], F32, kind="ExternalOutput").ap()
        dbg_mm83 = nc.dram_tensor("dbg_mm83", [J, 9], F32, kind="ExternalOutput").ap()

    rg = [list(range(N_CORES))]

    with tile.TileContext(nc) as tc:
        with tc.tile_pool(name="consts", bufs=1) as consts, \
             tc.tile_pool(name="sb", bufs=1) as sb, \
             tc.tile_pool(name="htp", bufs=2) as htp, \
             tc.tile_pool(name="psA", bufs=3, space="PSUM") as psA_pool, \
             tc.tile_pool(name="psJ", bufs=2, space="PSUM") as psJ_pool, \
             tc.tile_pool(name="misc", bufs=3, space="PSUM") as misc, \
             tc.tile_pool(name="dram", bufs=1, space="DRAM") as dram:

            # ---- constants / small inputs ----
            ident10 = consts.tile([NB, NB], F32, tag="id10")
            make_identity(nc, ident10[:])
            ones128 = consts.tile([128, 1], F32, tag="o128")
            nc.gpsimd.memset(ones128[:], 1.0)
            ones10 = consts.tile([NB, 1], F32, tag="o10")
            nc.gpsimd.memset(ones10[:], 1.0)
            ones1_10 = consts.tile([1, NB], F32, tag="o110")
            nc.gpsimd.memset(ones1_10[:], 1.0)
            ones8x8 = consts.tile([J, J], F32, tag="o88")
            nc.gpsimd.memset(ones8x8[:], 1.0)

            m_sb = consts.tile([128, NDC * J], F32, tag="msb")
            nc.gpsimd.dma_start(
                out=m_sb[:].rearrange("p (k j) -> p k j", j=J),
                in_=mmat.rearrange("(k p) j -> p k j", p=128))
            invc_sb = consts.tile([128, 1], F32, tag="invc")
            nc.gpsimd.dma_start(out=invc_sb[:], in_=invc[:])
            mem_sb = consts.tile([128, NDC * MEM], F32, tag="memsb")
            nc.gpsimd.dma_start(
                out=mem_sb[:].rearrange("p (k f) -> p k f", f=MEM),
                in_=memt.rearrange("(k p) f -> p k f", p=128))
            samp_sb = consts.tile([128, NDC * SN], F32, tag="sampsb")
            nc.gpsimd.dma_start(
                out=samp_sb[:].rearrange("p (k f) -> p k f", f=SN),
                in_=sampt.rearrange("(k p) f -> p k f", p=128))

            # ---- differentiation branch: Gram + row norms (early PE) ----
            psG = misc.tile([SN, SN], F32, tag="m")
            for k in range(NDC):
                nc.tensor.matmul(psG[:], samp_sb[:, k * SN:(k + 1) * SN],
                                 samp_sb[:, k * SN:(k + 1) * SN],
                                 start=(k == 0), stop=(k == NDC - 1))
            sqs = sb.tile([128, NDC * SN], F32, tag="sqs")
            nc.vector.tensor_tensor(sqs[:], samp_sb[:], samp_sb[:], OP.mult)
            psr = misc.tile([SN, 1], F32, tag="m")
            for k in range(NDC):
                nc.tensor.matmul(psr[:], sqs[:, k * SN:(k + 1) * SN],
                                 ones128[:], start=(k == 0),
                                 stop=(k == NDC - 1))
            g_sb = sb.tile([SN, SN], F32, tag="gsb")
            nc.scalar.copy(g_sb[:], psG[:])
            r_sb = sb.tile([SN, 1], F32, tag="rsb")
            nc.scalar.copy(r_sb[:], psr[:])

            # variance branch (DVE; overlaps stream)
            mem3 = mem_sb[:].rearrange("p (k f) -> p k f", f=MEM)
            mean16 = sb.tile([128, NDC], F32, tag="mean16")
            nc.vector.tensor_reduce(mean16[:], mem3, AX.X, OP.add)
            nc.vector.tensor_scalar(mean16[:], mean16[:], 1.0 / MEM, None,
                                    OP.mult)
            cent = sb.tile([128, NDC * MEM], F32, tag="cent")
            nc.vector.tensor_tensor(
                cent[:].rearrange("p (k f) -> p k f", f=MEM), mem3,
                mean16[:, :, None].broadcast_to([128, NDC, MEM]), OP.subtract)
            nc.vector.tensor_tensor(cent[:], cent[:], cent[:], OP.mult)
            var16 = sb.tile([128, NDC], F32, tag="var16")
            nc.vector.tensor_reduce(
                var16[:], cent[:].rearrange("p (k f) -> p k f", f=MEM),
                AX.X, OP.add)
            nc.vector.tensor_scalar(var16[:], var16[:], 1.0 / (MEM - 1), None,
                                    OP.mult)
            redv = sb.tile([128, 1], F32, tag="redv")
            nc.vector.tensor_reduce(redv[:], var16[:], AX.X, OP.add)
            v2 = sb.tile([128, NDC], F32, tag="v2")
            nc.vector.tensor_tensor(v2[:], var16[:], var16[:], OP.mult)
            redv2 = sb.tile([128, 1], F32, tag="redv2")
            nc.vector.tensor_reduce(redv2[:], v2[:], AX.X, OP.add)
            pstv = misc.tile([1, 1], F32, tag="m")
            nc.tensor.matmul(pstv[:], redv[:], ones128[:], start=True,
                             stop=True)
            tv_sb = sb.tile([1, 1], F32, tag="tvsb")
            nc.scalar.copy(tv_sb[:], pstv[:])
            pss2 = misc.tile([1, 1], F32, tag="m")
            nc.tensor.matmul(pss2[:], redv2[:], ones128[:], start=True,
                             stop=True)
            s2_sb = sb.tile([1, 1], F32, tag="s2sb")
            nc.scalar.copy(s2_sb[:], pss2[:])

            tvsq = sb.tile([1, 1], F32, tag="tvsq")
            nc.vector.tensor_tensor(tvsq[:], tv_sb[:], tv_sb[:], OP.mult)
            dden = sb.tile([1, 1], F32, tag="dden")
            nc.vector.scalar_tensor_tensor(dden[:], tvsq[:], 1e-6, s2_sb[:],
                                           OP.mult, OP.add)
            rdden = sb.tile([1, 1], F32, tag="rdden")
            nc.vector.reciprocal(rdden[:], dden[:])
            eff_sb = sb.tile([1, 1], F32, tag="effsb")
            nc.vector.tensor_tensor(eff_sb[:], tvsq[:], rdden[:], OP.mult)

            # cdist tail: d2 = r_i + r_j - 2G
            rrow_ps = misc.tile([1, SN], F32, tag="m")
            nc.tensor.transpose(rrow_ps[:], r_sb[:], ident10[:])
            rrow = sb.tile([1, SN], F32, tag="rrow")
            nc.scalar.copy(rrow[:], rrow_ps[:])
            rB = misc.tile([SN, SN], F32, tag="m")
            nc.tensor.matmul(rB[:], ones1_10[:], rrow[:], start=True,
                             stop=True)
            d2 = sb.tile([SN, SN], F32, tag="d2")
            nc.vector.scalar_tensor_tensor(d2[:], g_sb[:], -2.0, rB[:],
                                           OP.mult, OP.add)
            nc.vector.tensor_scalar(d2[:], d2[:], r_sb[:], 0.0, OP.add,
                                    OP.max)
            dst = sb.tile([SN, SN], F32, tag="dst")
            nc.scalar.activation(dst[:], d2[:], ACT.Sqrt)
            dsum = sb.tile([SN, 1], F32, tag="dsum")
            nc.vector.tensor_reduce(dsum[:], dst[:], AX.X, OP.add)
            psD = misc.tile([1, 1], F32, tag="m")
            nc.tensor.matmul(psD[:], dsum[:], ones10[:], start=True, stop=True)
            avg_sb = sb.tile([1, 1], F32, tag="avgsb")
            nc.vector.tensor_scalar(avg_sb[:], psD[:],
                                    float(1.0 / (SN * (SN - 1) + 1e-6)), None,
                                    OP.mult)
            sqtv = sb.tile([1, 1], F32, tag="sqtv")
            nc.scalar.activation(sqtv[:], tv_sb[:], ACT.Sqrt)
            diff_sb = sb.tile([1, 1], F32, tag="diffsb")
            nc.vector.tensor_tensor(diff_sb[:], sqtv[:], avg_sb[:], OP.mult)
            tanhd = sb.tile([1, 1], F32, tag="tanhd")
            nc.scalar.activation(tanhd[:], diff_sb[:], ACT.Tanh)

            # ---- stage A: stream HT, S.T = M.T @ HT into 3 packed banks ----
            psA = [psA_pool.tile([128, 512], F32, tag="sacc", name=f"psA{i}")
                   for i in range(3)]
            if variant in ("bigdma4", "bigdma8", "sim1big"):
                ng = 8 if variant == "bigdma8" else 4
                per = NDC // ng          # d-chunks per DMA group
                ht3 = ht.rearrange("(c p) t -> p c t", p=128)
                for g in range(ng):
                    htt = htp.tile([128, per * TL], F32, tag="htt",
                                   name="htt")
                    nc.sync.dma_start(
                        out=htt[:].rearrange("p (c t) -> p c t", t=TL),
                        in_=ht3[:, g * per:(g + 1) * per, :])
                    for ci in range(per):
                        dk = g * per + ci
                        for tcn in range(NTC):
                            b, q = ACC_MAP[tcn]
                            nc.tensor.matmul(
                                psA[b][32 * q:32 * q + J, :],
                                m_sb[:, dk * J:(dk + 1) * J],
                                htt[:, ci * TL + tcn * 512:
                                    ci * TL + (tcn + 1) * 512],
                                start=(dk == 0), stop=(dk == NDC - 1))
            else:
                sched = {"main": list(range(NDC)), "ndc1": [0],
                         "stream3": list(range(NDC)) * 3, "sim1": list(range(NDC)),
                         "multiq": list(range(NDC))}[variant]
                qengs = ([nc.sync, nc.scalar, nc.vector, nc.gpsimd]
                         if variant == "multiq" else [nc.sync])
                for i, dk in enumerate(sched):
                    htt = htp.tile([128, TL], F32, tag="htt", name="htt")
                    qengs[i % len(qengs)].dma_start(
                        out=htt[:], in_=ht[dk * 128:(dk + 1) * 128, :])
                    for tcn in range(NTC):
                        b, q = ACC_MAP[tcn]
                        nc.tensor.matmul(psA[b][32 * q:32 * q + J, :],
                                         m_sb[:, dk * J:(dk + 1) * J],
                                         htt[:, tcn * 512:(tcn + 1) * 512],
                                         start=(i == 0),
                                         stop=(i == len(sched) - 1))

            # ---- stage B: scale to SBUF (lane-aligned), min/max, AllReduce --
            # stS layout: [128, 3*512]; group (b,q): rows 32q..32q+7 hold
            # S.T rows for tcn=3b+q at cols b*512.., row 32q+8 = ones.
            stS = sb.tile([128, 3 * 512], F32, tag="sts")
            ones1536 = sb.tile([1, 3 * 512], F32, tag="ones1536")
            nc.vector.memset(ones1536[:], 1.0)
            for q in range(3):
                nc.sync.dma_start(out=stS[32 * q + J:32 * q + J + 1, :],
                                  in_=ones1536[:])
            mxb = sb.tile([128, 3], F32, tag="mxb")
            mnb = sb.tile([128, 3], F32, tag="mnb")
            nc.gpsimd.memset(mxb[:], -3.0e38)
            nc.gpsimd.memset(mnb[:], 3.0e38)
            for tcn in range(NTC):
                b, q = ACC_MAP[tcn]
                seg = stS[32 * q:32 * q + J, b * 512:(b + 1) * 512]
                nc.scalar.mul(seg, psA[b][32 * q:32 * q + J, :],
                              invc_sb[32 * q:32 * q + J, :])
                nc.vector.tensor_reduce(mxb[32 * q:32 * q + J, b:b + 1], seg,
                                        AX.X, OP.max)
                nc.vector.tensor_reduce(mnb[32 * q:32 * q + J, b:b + 1], seg,
                                        AX.X, OP.min)
            # gather lanes {32q+j} -> [8, 3] via SBUF->SBUF DMA remap
            mx83 = sb.tile([J, 9], F32, tag="mx83")
            mn83 = sb.tile([J, 9], F32, tag="mn83")
            for q in range(3):
                nc.sync.dma_start(out=mx83[:, 3 * q:3 * q + 3],
                                  in_=mxb[32 * q:32 * q + J, :])
                nc.sync.dma_start(out=mn83[:, 3 * q:3 * q + 3],
                                  in_=mnb[32 * q:32 * q + J, :])
            minmax = sb.tile([J, 2], F32, tag="minmax")
            nc.vector.tensor_reduce(minmax[:, 0:1], mx83[:], AX.X, OP.max)
            tmn = sb.tile([J, 1], F32, tag="tmn")
            nc.vector.tensor_reduce(tmn[:], mn83[:], AX.X, OP.min)
            nc.vector.tensor_scalar(minmax[:, 1:2], tmn[:], -1.0, None,
                                    OP.mult)
            cbA = dram.tile([J, 2], F32, tag="cba")
            cbB = dram.tile([J, 2], F32, tag="cbb")
            nc.gpsimd.dma_start(out=cbA[:], in_=minmax[:])
            if sim1:
                nc.gpsimd.dma_start(out=cbB[:], in_=cbA[:])
            else:
                nc.gpsimd.collective_compute("AllReduce", OP.max,
                                             replica_groups=rg,
                                             ins=[cbA.opt()],
                                             outs=[cbB.opt()])
            gmm = sb.tile([J, 2], F32, tag="gmm")
            nc.gpsimd.dma_start(out=gmm[:], in_=cbB[:])

            # s1 = 10/(max-min+1e-6); b1 = -min*s1 - 0.5 (RNE cast -> floor)
            gmn = sb.tile([J, 1], F32, tag="gmn")
            nc.vector.tensor_scalar(gmn[:], gmm[:, 1:2], -1.0, None, OP.mult)
            dden2 = sb.tile([J, 1], F32, tag="dden2")
            nc.vector.tensor_tensor(dden2[:], gmm[:, 0:1], gmn[:], OP.subtract)
            nc.vector.tensor_scalar(dden2[:], dden2[:], 1e-6, None, OP.add)
            rdd = sb.tile([J, 1], F32, tag="rdd")
            nc.vector.reciprocal(rdd[:], dden2[:])
            s1 = sb.tile([J, 1], F32, tag="s1")
            nc.vector.tensor_scalar(s1[:], rdd[:], 10.0, None, OP.mult)
            b1 = sb.tile([J, 1], F32, tag="b1")
            nc.vector.tensor_tensor(b1[:], gmn[:], s1[:], OP.mult)
            nc.vector.tensor_scalar(b1[:], b1[:], -1.0, -0.5, OP.mult, OP.add)

            # R [9,8] replicated at partition bases 0/32/64:
            # rows 32q..32q+7 diag(s1), row 32q+8 = b1 row
            s1b = sb.tile([J, J], F32, tag="s1b")
            nc.vector.tensor_scalar(s1b[:], ones8x8[:], s1[:], None, OP.mult)
            rmat = sb.tile([128, J], F32, tag="rmat")
            nc.gpsimd.memset(rmat[:], 0.0)
            nc.gpsimd.affine_select(out=rmat[0:J, :], in_=s1b[:],
                                    compare_op=OP.is_equal, fill=0.0, base=0,
                                    pattern=[[-1, J]], channel_multiplier=1)
            nc.sync.dma_start(out=rmat[J:J + 1, 0:J], in_=b1[:])
            nc.sync.dma_start(out=rmat[32:32 + J + 1, :], in_=rmat[0:J + 1, :])
            nc.sync.dma_start(out=rmat[64:64 + J + 1, :], in_=rmat[0:J + 1, :])

            # ---- stage C: affine+transpose via PE, bin, one-hot, joints ----
            psC = misc.tile([128, NCH * J], F32, tag="m")
            for tcn in range(NTC):
                b, q = ACC_MAP[tcn]
                for c in range(4):
                    gc = tcn * 4 + c
                    nc.tensor.matmul(
                        psC[:, gc * J:(gc + 1) * J],
                        stS[32 * q:32 * q + J + 1,
                            b * 512 + c * 128:b * 512 + (c + 1) * 128],
                        rmat[32 * q:32 * q + J + 1, :],
                        start=True, stop=True)
            binint = sb.tile([128, NCH * J], I32, tag="binint")
            nc.vector.tensor_copy(binint[:], psC[:])
            nc.vector.tensor_scalar(binint[:], binint[:], 0, NB - 1, OP.max,
                                    OP.min)
            ohsb = sb.tile([128, NCH * J * NB], F32, tag="ohsb")
            oh3 = ohsb[:].rearrange("p (c b) -> p c b", b=NB)
            for b in range(NB):
                nc.vector.tensor_scalar(oh3[:, :, b], binint[:], b, None,
                                        OP.is_equal)
            # joint histograms: psJt1 packs pairs 0..2 at bases 0/32/64
            psJt1 = psJ_pool.tile([128, NB], F32, tag="pj", name="psJt1")
            psJt2 = psJ_pool.tile([NB, NB], F32, tag="pj", name="psJt2")
            for p in range(NPAIR):
                outap = (psJt2[:] if p == 3
                         else psJt1[32 * p:32 * p + NB, :])
                for c in range(NCH):
                    xa = (c * J + 2 * p) * NB
                    ya = (c * J + 2 * p + 1) * NB
                    nc.tensor.matmul(outap, ohsb[:, xa:xa + NB],
                                     ohsb[:, ya:ya + NB], start=(c == 0),
                                     stop=(c == NCH - 1))
            jm1 = sb.tile([128, NB], F32, tag="jm1")
            jm2 = sb.tile([NB, NB], F32, tag="jm2")
            for p in range(3):
                nc.scalar.copy(jm1[32 * p:32 * p + NB, :],
                               psJt1[32 * p:32 * p + NB, :])
            nc.scalar.copy(jm2[:], psJt2[:])
            cbj = dram.tile([NPAIR, NB * NB], F32, tag="cbj")
            cbj2 = dram.tile([NPAIR, NB * NB], F32, tag="cbj2")
            for p in range(3):
                nc.gpsimd.dma_start(
                    out=cbj[p:p + 1, :],
                    in_=jm1[32 * p:32 * p + NB, :])
            nc.gpsimd.dma_start(out=cbj[3:4, :], in_=jm2[:])
            if sim1:
                nc.gpsimd.dma_start(out=cbj2[:], in_=cbj[:])
            else:
                nc.gpsimd.collective_compute("AllReduce", OP.add,
                                             replica_groups=rg,
                                             ins=[cbj.opt()],
                                             outs=[cbj2.opt()])
            gj = sb.tile([NB, NPAIR * NB], F32, tag="gj")
            nc.gpsimd.dma_start(
                out=gj[:].rearrange("a (p b) -> a p b", b=NB),
                in_=cbj2[:].rearrange("p (a b) -> a p b", a=NB))

            # ---- stage D: MI per pair ----
            mirow = sb.tile([1, NPAIR], F32, tag="mirow")
            for p in range(NPAIR):
                gjp = gj[:, p * NB:(p + 1) * NB]
                rowsum = sb.tile([NB, 1], F32, tag="rowsum", name="rowsum")
                nc.vector.tensor_reduce(rowsum[:], gjp, AX.X, OP.add)
                colps = misc.tile([NB, 1], F32, tag="m", name="colps")
                nc.tensor.matmul(colps[:], gjp, ones10[:], start=True,
                                 stop=True)
                totps = misc.tile([1, 1], F32, tag="m", name="totps")
                nc.tensor.matmul(totps[:], rowsum[:], ones10[:], start=True,
                                 stop=True)
                tot = sb.tile([1, 1], F32, tag="tot", name="tot")
                nc.vector.tensor_scalar(tot[:], totps[:], 1e-10, None, OP.add)
                tinv = sb.tile([1, 1], F32, tag="tinv", name="tinv")
                nc.vector.reciprocal(tinv[:], tot[:])
                t10ps = misc.tile([NB, 1], F32, tag="m", name="t10ps")
                nc.tensor.matmul(t10ps[:], ones1_10[:], tinv[:], start=True,
                                 stop=True)
                t10 = sb.tile([NB, 1], F32, tag="t10", name="t10")
                nc.scalar.copy(t10[:], t10ps[:])
                jn = sb.tile([NB, NB], F32, tag="jn", name="jn")
                nc.vector.tensor_scalar(jn[:], gjp, t10[:], None, OP.mult)
                px = sb.tile([NB, 1], F32, tag="px", name="px")
                nc.vector.tensor_scalar(px[:], rowsum[:], t10[:], None,
                                        OP.mult)
                py = sb.tile([NB, 1], F32, tag="py", name="py")
                nc.vector.tensor_scalar(py[:], colps[:], t10[:], None,
                                        OP.mult)
                pyr_ps = misc.tile([1, NB], F32, tag="m", name="pyr_ps")
                nc.tensor.transpose(pyr_ps[:], py[:], ident10[:])
                pyr = sb.tile([1, NB], F32, tag="pyr", name="pyr")
                nc.scalar.copy(pyr[:], pyr_ps[:])
                pyB = misc.tile([NB, NB], F32, tag="m", name="pyB")
                nc.tensor.matmul(pyB[:], ones1_10[:], pyr[:], start=True,
                                 stop=True)
                outer = sb.tile([NB, NB], F32, tag="outer", name="outer")
                nc.vector.tensor_scalar(outer[:], pyB[:], px[:], None,
                                        OP.mult)
                num = sb.tile([NB, NB], F32, tag="num", name="num")
                nc.vector.tensor_scalar(num[:], jn[:], 1e-10, None, OP.add)
                nc.vector.tensor_scalar(outer[:], outer[:], 1e-10, None,
                                        OP.add)
                rout = sb.tile([NB, NB], F32, tag="rout", name="rout")
                nc.vector.reciprocal(rout[:], outer[:])
                nc.vector.tensor_tensor(num[:], num[:], rout[:], OP.mult)
                lg = sb.tile([NB, NB], F32, tag="lg", name="lg")
                nc.scalar.activation(lg[:], num[:], ACT.Ln)
                nc.vector.tensor_tensor(lg[:], jn[:], lg[:], OP.mult)
                ms = sb.tile([NB, 1], F32, tag="ms", name="ms")
                nc.vector.tensor_reduce(ms[:], lg[:], AX.X, OP.add)
                mips = misc.tile([1, 1], F32, tag="m", name="mips")
                nc.tensor.matmul(mips[:], ms[:], ones10[:], start=True,
                                 stop=True)
                nc.vector.tensor_scalar(mirow[:, p:p + 1], mips[:], 0.0, None,
                                        OP.max)

            integ = sb.tile([1, 1], F32, tag="integ")
            nc.vector.tensor_reduce(integ[:], mirow[:], AX.X, OP.min)
            consc = sb.tile([1, 1], F32, tag="consc")
            nc.vector.tensor_tensor(consc[:], integ[:], tanhd[:], OP.add)

            outrow = sb.tile([1, 9], F32, tag="outrow")
            nc.vector.tensor_copy(outrow[:, 0:1], consc[:])
            nc.vector.tensor_copy(outrow[:, 1:2], diff_sb[:])
            nc.vector.tensor_copy(outrow[:, 2:3], eff_sb[:])
            nc.vector.tensor_copy(outrow[:, 3:4], tv_sb[:])
            nc.vector.tensor_copy(outrow[:, 4:5], integ[:])
            nc.vector.tensor_copy(outrow[:, 5:9], mirow[:])
            nc.sync.dma_start(out=out[:], in_=outrow[:])
            if debug:
                nc.sync.dma_start(out=dbg_st[:], in_=stS[0:J, 0:512])
                nc.sync.dma_start(out=dbg_gmm[:], in_=gmm[:])
                nc.sync.dma_start(out=dbg_rmat[:], in_=rmat[0:J + 1, :])
                nc.sync.dma_start(out=dbg_bin[:], in_=binint[:, 0:16])
                nc.sync.dma_start(out=dbg_gj[:], in_=gj[:])
                nc.sync.dma_start(out=dbg_mm83[:], in_=mx83[:])

    nc.compile()
    return nc


def _build_variant(name):
    return _build(variant=name)


def _get_nc(debug=False):
    key = ("ncd" if debug else "nc")
    if key not in _CACHE:
        _CACHE[key] = _build(debug)
    return _CACHE[key]


def kernel(state, state_memory, state_history, partitions, sample_idx,
           trace=False, debug=False):
    global LAST_RESULTS
    state = np.asarray(state, np.float32)
    state_memory = np.asarray(state_memory, np.float32)
    state_history = np.asarray(state_history, np.float32)
    partitions = np.asarray(partitions)
    sample_idx = np.asarray(sample_idx)

    mmat = np.empty((D, J), np.float32)
    invc8 = np.empty((J,), np.float32)
    pf = partitions.astype(np.float32)
    for p in range(NPAIR):
        mmat[:, 2 * p] = pf[p]
        mmat[:, 2 * p + 1] = np.float32(1.0) - pf[p]
        invc8[2 * p] = np.float32(1.0) / pf[p].sum(dtype=np.float32)
        invc8[2 * p + 1] = np.float32(1.0) / (np.float32(1.0) - pf[p]).sum(
            dtype=np.float32)
    invc = np.zeros((128, 1), np.float32)
    for q in range(3):
        invc[32 * q:32 * q + J, 0] = invc8
    memory = np.concatenate([state, state_memory[state.shape[0]:]], axis=0)
    memt = np.ascontiguousarray(memory.T)
    sampt = np.ascontiguousarray(memory[sample_idx].T)

    in_maps = []
    for c in range(N_CORES):
        htc = np.ascontiguousarray(state_history[c * TL:(c + 1) * TL, :].T)
        in_maps.append({"ht": htc, "mmat": mmat, "invc": invc,
                        "memt": memt, "sampt": sampt})

    nc = _get_nc(debug)
    res = run_bass_kernel_spmd(nc, in_maps, list(range(N_CORES)),
                               trace=trace)
    LAST_RESULTS = res
    return np.asarray(res.results[0]["out"], np.float32)



# revision 16
# speedup vs baseline: 2.1756x; 2.1756x over previous
"""Trainium2 Bass kernel for nn_ConsciousnessMonitor (histogram_binning).

kernel(**inputs) takes FULL unsharded numpy inputs, returns the full (9,)
float32 output. Shards state_history along time across 8 NeuronCores.

Design: state_history streamed as fp16 (halves HBM traffic; empirically
max-rel-err from the cast is 6.9e-3 vs the 2e-2 gate). The masked-mean
matmul runs transposed (H chunk stationary, 8 mask columns moving) so the
projections land time-on-partitions in a single PSUM region [128, 32*8];
that removes the affine/transpose stage entirely. Min/max reductions split
across DVE and Pool, then a 64B AllReduce of (max,-min); binning is two
DVE ops + 10 one-hot compares (DVE/Pool split) + 32 packed 40x40 joint
matmuls; a 1.6KB AllReduce of the 4 joint histograms; the MI tail is
batched across all 4 pairs (pairs stacked along partitions).

Self-contained: shapes/sharding hardcoded; reads no sibling files.
"""
import numpy as np

import concourse.bacc as bacc
import concourse.tile as tile
import concourse.mybir as mybir
from concourse.bass_utils import run_bass_kernel_spmd

F32 = mybir.dt.float32
F16 = mybir.dt.float16
I16 = mybir.dt.int16
AX = mybir.AxisListType
OP = mybir.AluOpType
ACT = mybir.ActivationFunctionType

N_CORES = 8
T, D = 32768, 2048
TL = T // N_CORES          # 4096 time steps per core
NB = 10                    # histogram bins per axis
NPAIR = 4                  # partitions (mask pairs)
J = 2 * NPAIR              # 8 masked-sum series (x0..x3, y0..y3)
NDC = D // 128             # 16 contraction chunks
NCH = TL // 128            # 32 time chunks of 128 (partition-major)
MEM = 100
SN = 10

# DMA segments: (dk, t0, t1); last chunk split so its matmul tail is short
SEGS = [(dk, 0, TL) for dk in range(NDC - 1)]
SEGS += [(NDC - 1, 0, TL // 2), (NDC - 1, TL // 2, TL)]

_CACHE = {}
LAST_RESULTS = None


def _build(debug=False, variant="main"):
    sim1 = variant.startswith("sim1")
    nc = bacc.Bacc("TRN2", target_bir_lowering=False, debug=False,
                   num_devices=1 if sim1 else N_CORES)
    ht = nc.dram_tensor("ht", [D, TL], F16, kind="ExternalInput").ap()
    m8 = nc.dram_tensor("m8", [D, J], F16, kind="ExternalInput").ap()
    memt = nc.dram_tensor("memt", [D, MEM], F16, kind="ExternalInput").ap()
    sampt = nc.dram_tensor("sampt", [D, SN], F16, kind="ExternalInput").ap()
    # cst1 rows 0:40 cols 0:4 = blkones; row 0: cols 4:12 = invc,
    # cols 12:20 = 10*invc
    cst1 = nc.dram_tensor("cst1", [40, 20], F32, kind="ExternalInput").ap()
    # cst2: [4, 44] = blkT [4,40] | ident4 [4,4]
    cst2 = nc.dram_tensor("cst2", [4, 44], F32, kind="ExternalInput").ap()
    out = nc.dram_tensor("out", [9], F32, kind="ExternalOutput").ap()
    if debug:
        dbg_s = nc.dram_tensor("dbg_s", [128, 16], F32, kind="ExternalOutput").ap()
        dbg_mm = nc.dram_tensor("dbg_mm", [1, 16], F32, kind="ExternalOutput").ap()
        dbg_bin = nc.dram_tensor("dbg_bin", [128, 16], I16, kind="ExternalOutput").ap()
        dbg_jnt = nc.dram_tensor("dbg_jnt", [40, 10], F32, kind="ExternalOutput").ap()
        dbg_mi = nc.dram_tensor("dbg_mi", [1, 9], F32, kind="ExternalOutput").ap()

    rg = [list(range(N_CORES))]

    with tile.TileContext(nc) as tc:
        with tc.tile_pool(name="consts", bufs=1) as consts, \
             tc.tile_pool(name="sb", bufs=1) as sb, \
             tc.tile_pool(name="htp", bufs=4) as htp, \
             tc.tile_pool(name="psS", bufs=1, space="PSUM") as psS_pool, \
             tc.tile_pool(name="psJ", bufs=4, space="PSUM") as psJ_pool, \
             tc.tile_pool(name="misc", bufs=3, space="PSUM") as misc, \
             tc.tile_pool(name="dram", bufs=1, space="DRAM") as dram:

            # ---- mask matrix first: gates the stream matmuls ----
            m_sb = consts.tile([128, NDC * J], F16, tag="msb")
            nc.scalar.dma_start(
                out=m_sb[:].rearrange("p (k j) -> p k j", j=J),
                in_=m8.rearrange("(k p) j -> p k j", p=128))
            ones128 = consts.tile([128, 1], F32, tag="o128")
            nc.gpsimd.memset(ones128[:], 1.0)
            ones10 = consts.tile([SN, 1], F32, tag="o10")
            nc.gpsimd.memset(ones10[:], 1.0)
            ones1_10 = consts.tile([1, SN], F32, tag="o110")
            nc.vector.memset(ones1_10[:], 1.0)

            # ---- stream: psS[t128, (tc, j)] += ht_chunk.T @ m ----
            psS = psS_pool.tile([128, NCH * J], F32, tag="psS")
            for si, (dk, t0, t1) in enumerate(SEGS):
                htt = htp.tile([128, t1 - t0], F16, tag="htt", name="htt")
                nc.sync.dma_start(out=htt[:], in_=ht[dk * 128:(dk + 1) * 128,
                                                     t0:t1])
                for tcn in range(t0 // 128, t1 // 128):
                    # start=True zeroes the whole 2KB PSUM region, so only
                    # the very first matmul starts; only the last stops
                    nc.tensor.matmul(
                        psS[:, tcn * J:(tcn + 1) * J],
                        htt[:, tcn * 128 - t0:(tcn + 1) * 128 - t0],
                        m_sb[:, dk * J:(dk + 1) * J],
                        start=(si == 0 and tcn == 0),
                        stop=(si == len(SEGS) - 1 and tcn == NCH - 1))

            # ---- small inputs: issued late so their DMA slots fall after
            # the stream on the shared DMA engines; consumers have slack ----
            cst1_sb = consts.tile([40, 20], F32, tag="cst1")
            nc.scalar.dma_start(out=cst1_sb[:], in_=cst1[:])
            cst2_sb = consts.tile([4, 44], F32, tag="cst2")
            nc.scalar.dma_start(out=cst2_sb[:], in_=cst2[:])
            blkA = cst1_sb[0:40, 0:4]       # [40,4] block indicator
            invc_row = cst1_sb[0:1, 4:12]   # [1,8] 1/count
            invc10_row = cst1_sb[0:1, 12:20]  # [1,8] 10/count
            blkT = cst2_sb[:, 0:40]         # [4,40]
            id4 = cst2_sb[:, 40:44]         # [4,4]
            mem_sb = consts.tile([128, NDC * MEM], F16, tag="memsb")
            nc.scalar.dma_start(
                out=mem_sb[:].rearrange("p (k f) -> p k f", f=MEM),
                in_=memt.rearrange("(k p) f -> p k f", p=128))
            samp_sb = consts.tile([128, NDC * SN], F16, tag="sampsb")
            nc.scalar.dma_start(
                out=samp_sb[:].rearrange("p (k f) -> p k f", f=SN),
                in_=sampt.rearrange("(k p) f -> p k f", p=128))

            # ---- min/max over local t (DVE max || Pool min), AllReduce ----
            ps3 = psS[:].rearrange("p (c j) -> p j c", j=J)
            pboth = sb.tile([128, 2 * J], F32, tag="pboth")
            nc.vector.tensor_reduce(pboth[:, 0:J], ps3, AX.X, OP.max)
            nc.vector.tensor_reduce(pboth[:, J:2 * J], ps3, AX.X, OP.min)
            nc.vector.tensor_scalar(pboth[:, J:2 * J], pboth[:, J:2 * J],
                                    -1.0, None, OP.mult)
            arb = sb.tile([128, 2 * J], F32, tag="arb")
            nc.gpsimd.partition_all_reduce(arb[:], pboth[:], 128,
                                           mybir_reduce_max())
            cbA = dram.tile([1, 2 * J], F32, tag="cba")
            cbB = dram.tile([1, 2 * J], F32, tag="cbb")
            nc.sync.dma_start(out=cbA[:], in_=arb[0:1, :])
            gmm = sb.tile([1, 2 * J], F32, tag="gmm")
            if sim1:
                nc.scalar.dma_start(out=gmm[:], in_=cbA[:])
            else:
                nc.gpsimd.collective_compute("AllReduce", OP.max,
                                             replica_groups=rg,
                                             ins=[cbA.opt()],
                                             outs=[cbB.opt()])
                nc.scalar.dma_start(out=gmm[:], in_=cbB[:])

            # ---- differentiation branch (fills the AllReduce wait) ----
            outrow = sb.tile([1, 9], F32, tag="outrow")
            psG = misc.tile([SN, SN], F32, tag="m", name="psG")
            for k in range(NDC):
                nc.tensor.matmul(psG[:], samp_sb[:, k * SN:(k + 1) * SN],
                                 samp_sb[:, k * SN:(k + 1) * SN],
                                 start=(k == 0), stop=(k == NDC - 1))
            sqs = sb.tile([128, NDC * SN], F32, tag="sqs")
            nc.vector.tensor_tensor(sqs[:], samp_sb[:], samp_sb[:], OP.mult)
            psrc = misc.tile([SN, 1], F32, tag="m", name="psrc")
            psrr = misc.tile([1, SN], F32, tag="m", name="psrr")
            for k in range(NDC):
                nc.tensor.matmul(psrc[:], sqs[:, k * SN:(k + 1) * SN],
                                 ones128[:], start=(k == 0),
                                 stop=(k == NDC - 1))
                nc.tensor.matmul(psrr[:], ones128[:],
                                 sqs[:, k * SN:(k + 1) * SN], start=(k == 0),
                                 stop=(k == NDC - 1))
            g_sb = sb.tile([SN, SN], F32, tag="gsb")
            nc.scalar.copy(g_sb[:], psG[:])
            rcol = sb.tile([SN, 1], F32, tag="rcol")
            nc.scalar.copy(rcol[:], psrc[:])
            rrow = sb.tile([1, SN], F32, tag="rrow")
            nc.scalar.copy(rrow[:], psrr[:])

            mem3 = mem_sb[:].rearrange("p (k f) -> p k f", f=MEM)
            mean16 = sb.tile([128, NDC], F32, tag="mean16")
            nc.vector.tensor_reduce(mean16[:], mem3, AX.X, OP.add)
            nc.vector.tensor_scalar(mean16[:], mean16[:], 1.0 / MEM, None,
                                    OP.mult)
            cent = sb.tile([128, NDC * MEM], F32, tag="cent")
            nc.vector.tensor_tensor(
                cent[:].rearrange("p (k f) -> p k f", f=MEM), mem3,
                mean16[:, :, None].broadcast_to([128, NDC, MEM]), OP.subtract)
            nc.vector.tensor_tensor(cent[:], cent[:], cent[:], OP.mult)
            var16 = sb.tile([128, NDC], F32, tag="var16")
            nc.vector.tensor_reduce(
                var16[:], cent[:].rearrange("p (k f) -> p k f", f=MEM),
                AX.X, OP.add)
            nc.vector.tensor_scalar(var16[:], var16[:], 1.0 / (MEM - 1), None,
                                    OP.mult)
            redv = sb.tile([128, 1], F32, tag="redv")
            nc.vector.tensor_reduce(redv[:], var16[:], AX.X, OP.add)
            v2 = sb.tile([128, NDC], F32, tag="v2")
            nc.vector.tensor_tensor(v2[:], var16[:], var16[:], OP.mult)
            redv2 = sb.tile([128, 1], F32, tag="redv2")
            nc.vector.tensor_reduce(redv2[:], v2[:], AX.X, OP.add)
            pstv = misc.tile([1, 1], F32, tag="m", name="pstv")
            nc.tensor.matmul(pstv[:], redv[:], ones128[:], start=True,
                             stop=True)
            tv_sb = outrow[:, 3:4]
            nc.scalar.copy(tv_sb, pstv[:])
            pss2 = misc.tile([1, 1], F32, tag="m", name="pss2")
            nc.tensor.matmul(pss2[:], redv2[:], ones128[:], start=True,
                             stop=True)
            s2_sb = sb.tile([1, 1], F32, tag="s2sb")
            nc.scalar.copy(s2_sb[:], pss2[:])

            tvsq = sb.tile([1, 1], F32, tag="tvsq")
            nc.vector.tensor_tensor(tvsq[:], tv_sb, tv_sb, OP.mult)
            dden = sb.tile([1, 1], F32, tag="dden")
            nc.vector.scalar_tensor_tensor(dden[:], tvsq[:], 1e-6, s2_sb[:],
                                           OP.mult, OP.add)
            rdden = sb.tile([1, 1], F32, tag="rdden")
            nc.vector.reciprocal(rdden[:], dden[:])
            nc.vector.tensor_tensor(outrow[:, 2:3], tvsq[:], rdden[:],
                                    OP.mult)

            # cdist tail: d2 = r_i + r_j - 2G
            rB = misc.tile([SN, SN], F32, tag="m", name="rB")
            nc.tensor.matmul(rB[:], ones1_10[:], rrow[:], start=True,
                             stop=True)
            d2 = sb.tile([SN, SN], F32, tag="d2")
            nc.vector.scalar_tensor_tensor(d2[:], g_sb[:], -2.0, rB[:],
                                           OP.mult, OP.add)
            nc.vector.tensor_scalar(d2[:], d2[:], rcol[:], 0.0, OP.add,
                                    OP.max)
            dst = sb.tile([SN, SN], F32, tag="dst")
            nc.scalar.activation(dst[:], d2[:], ACT.Sqrt)
            dsum = sb.tile([SN, 1], F32, tag="dsum")
            nc.vector.tensor_reduce(dsum[:], dst[:], AX.X, OP.add)
            psD = misc.tile([1, 1], F32, tag="m", name="psD")
            nc.tensor.matmul(psD[:], dsum[:], ones10[:], start=True, stop=True)
            avg_sb = sb.tile([1, 1], F32, tag="avgsb")
            nc.vector.tensor_scalar(avg_sb[:], psD[:],
                                    float(1.0 / (SN * (SN - 1) + 1e-6)), None,
                                    OP.mult)
            sqtv = sb.tile([1, 1], F32, tag="sqtv")
            nc.scalar.activation(sqtv[:], tv_sb, ACT.Sqrt)
            diff_sb = outrow[:, 1:2]
            nc.vector.tensor_tensor(diff_sb, sqtv[:], avg_sb[:], OP.mult)
            tanhd = sb.tile([1, 1], F32, tag="tanhd")
            nc.scalar.activation(tanhd[:], diff_sb, ACT.Tanh)
            # preload the Ln table during collective slack so the MI-tail
            # Ln doesn't pay the 1.3us table switch (copies stay in-set)
            lnwarm = sb.tile([1, 1], F32, tag="lnwarm")
            nc.scalar.activation(lnwarm[:], tanhd[:], ACT.Ln)

            # ---- bin coefficients from global raw (max | -min) ----
            # v = raw/cnt; s = 10/((max-min)/cnt + 1e-6)
            # s1 = s/cnt ; b1 = (-min/cnt)*s - 0.5  (RNE of x-0.5 = floor)
            rng = sb.tile([1, J], F32, tag="rng")
            nc.vector.tensor_tensor(rng[:], gmm[:, 0:J], gmm[:, J:2 * J],
                                    OP.add)
            den = sb.tile([1, J], F32, tag="den")
            nc.vector.tensor_tensor(den[:], rng[:], invc_row, OP.mult)
            nc.vector.tensor_scalar(den[:], den[:], 1e-6, None, OP.add)
            rden = sb.tile([1, J], F32, tag="rden")
            nc.vector.reciprocal(rden[:], den[:])
            coefr = sb.tile([1, 2 * J], F32, tag="coefr")
            nc.vector.tensor_tensor(coefr[:, 0:J], rden[:], invc10_row,
                                    OP.mult)
            nm10 = sb.tile([1, J], F32, tag="nm10")
            nc.vector.tensor_tensor(nm10[:], gmm[:, J:2 * J], invc10_row,
                                    OP.mult)
            nc.vector.tensor_tensor(coefr[:, J:2 * J], nm10[:], rden[:],
                                    OP.mult)
            nc.vector.tensor_scalar(coefr[:, J:2 * J], coefr[:, J:2 * J],
                                    -0.5, None, OP.add)
            coef = sb.tile([128, 2 * J], F32, tag="coef")
            nc.gpsimd.partition_broadcast(coef[:], coefr[:])

            # ---- binning: binint = RNE(psS*s1 + b1) as int16 ----
            binf = sb.tile([128, NCH * J], F32, tag="binf")
            bf3 = binf[:].rearrange("p (c j) -> p c j", j=J)
            nc.vector.tensor_tensor(
                bf3, psS[:].rearrange("p (c j) -> p c j", j=J),
                coef[:, None, 0:J].broadcast_to([128, NCH, J]), OP.mult)
            binint = sb.tile([128, NCH * J], I16, tag="binint")
            nc.vector.tensor_tensor(
                binint[:].rearrange("p (c j) -> p c j", j=J), bf3,
                coef[:, None, J:2 * J].broadcast_to([128, NCH, J]), OP.add)

            # ---- one-hot (7 bins on DVE, 3 on Pool; edge bins clamp) ----
            ohsb = sb.tile([128, NCH * J * NB], F16, tag="ohsb")
            oh4 = ohsb[:].rearrange("p (c j b) -> p c j b", j=J, b=NB)
            bi3 = binint[:].rearrange("p (c j) -> p c j", j=J)
            for b in range(NB):
                eng = nc.vector if b < 7 else nc.gpsimd
                op = OP.is_le if b == 0 else (OP.is_ge if b == NB - 1
                                              else OP.is_equal)
                eng.tensor_scalar(oh4[:, :, :, b], bi3, b, None, op)

            # ---- joint histograms: pair p -> its own PSUM bank tile ----
            psJp = [psJ_pool.tile([NB, NB], F32, tag="psJ", name=f"psJ{p}")
                    for p in range(NPAIR)]
            for c in range(NCH):
                base = c * J * NB
                for p in range(NPAIR):
                    nc.tensor.matmul(
                        psJp[p][:],
                        ohsb[:, base + NB * p:base + NB * (p + 1)],
                        ohsb[:, base + 40 + NB * p:base + 40 + NB * (p + 1)],
                        start=(c == 0), stop=(c == NCH - 1))
            jm = sb.tile([NB, NPAIR * NB], F32, tag="jm")
            nc.scalar.copy(jm[:, 0:NB], psJp[0][:])
            nc.vector.tensor_copy(jm[:, NB:2 * NB], psJp[1][:])
            nc.scalar.copy(jm[:, 2 * NB:3 * NB], psJp[2][:])
            nc.vector.tensor_copy(jm[:, 3 * NB:4 * NB], psJp[3][:])
            cbj = dram.tile([NPAIR, NB * NB], F32, tag="cbj")
            cbj2 = dram.tile([NPAIR, NB * NB], F32, tag="cbj2")
            nc.sync.dma_start(
                out=cbj[:].rearrange("p (a b) -> a p b", a=NB),
                in_=jm[:].rearrange("a (p b) -> a p b", b=NB))
            jnt = sb.tile([40, NB], F32, tag="jnt")
            if sim1:
                nc.sync.dma_start(
                    out=jnt[:], in_=cbj.rearrange("p (a b) -> (p a) b", a=NB))
            else:
                nc.gpsimd.collective_compute("AllReduce", OP.add,
                                             replica_groups=rg,
                                             ins=[cbj.opt()],
                                             outs=[cbj2.opt()])
                nc.sync.dma_start(
                    out=jnt[:], in_=cbj2.rearrange("p (a b) -> (p a) b", a=NB))

            # ---- MI, batched across pairs (pairs along partitions) ----
            rowsum = sb.tile([40, 1], F32, tag="rowsum")
            nc.vector.tensor_reduce(rowsum[:], jnt[:], AX.X, OP.add)
            pscol = misc.tile([NPAIR, NB], F32, tag="m", name="pscol")
            nc.tensor.matmul(pscol[:], blkA, jnt[:], start=True, stop=True)
            tot4 = sb.tile([NPAIR, 1], F32, tag="tot4")
            nc.vector.tensor_reduce(tot4[:], pscol[:], AX.X, OP.add)
            tinv4 = sb.tile([NPAIR, 1], F32, tag="tinv4")
            nc.vector.reciprocal(tinv4[:], tot4[:])
            pst40 = misc.tile([40, 1], F32, tag="m", name="pst40")
            nc.tensor.matmul(pst40[:], blkT, tinv4[:], start=True, stop=True)
            tinv40 = sb.tile([40, 1], F32, tag="tinv40")
            nc.vector.tensor_copy(tinv40[:], pst40[:])
            px = sb.tile([40, 1], F32, tag="px")
            nc.vector.tensor_tensor(px[:], rowsum[:], tinv40[:], OP.mult)
            py4 = sb.tile([NPAIR, NB], F32, tag="py4")
            nc.vector.tensor_scalar(py4[:], pscol[:], tinv4[:], None, OP.mult)
            pspy = misc.tile([40, NB], F32, tag="m", name="pspy")
            nc.tensor.matmul(pspy[:], blkT, py4[:], start=True, stop=True)
            jn = sb.tile([40, NB], F32, tag="jn")
            nc.vector.tensor_scalar(jn[:], jnt[:], tinv40[:], None, OP.mult)
            outer = sb.tile([40, NB], F32, tag="outer")
            nc.vector.tensor_scalar(outer[:], pspy[:], px[:], 1e-10, OP.mult,
                                    OP.add)
            num = sb.tile([40, NB], F32, tag="num")
            nc.vector.tensor_scalar(num[:], jn[:], 1e-10, None, OP.add)
            rout = sb.tile([40, NB], F32, tag="rout")
            nc.vector.reciprocal(rout[:], outer[:])
            nc.vector.tensor_tensor(num[:], num[:], rout[:], OP.mult)
            lg = sb.tile([40, NB], F32, tag="lg")
            nc.scalar.activation(lg[:], num[:], ACT.Ln)
            nc.vector.tensor_tensor(lg[:], jn[:], lg[:], OP.mult)
            ms = sb.tile([40, 1], F32, tag="ms")
            nc.vector.tensor_reduce(ms[:], lg[:], AX.X, OP.add)
            psmi = misc.tile([NPAIR, 1], F32, tag="m", name="psmi")
            nc.tensor.matmul(psmi[:], blkA, ms[:], start=True, stop=True)
            mirow4 = sb.tile([NPAIR, 1], F32, tag="mirow4")
            nc.vector.tensor_scalar(mirow4[:], psmi[:], 0.0, None, OP.max)
            psmT = misc.tile([1, NPAIR], F32, tag="m", name="psmT")
            nc.tensor.matmul(psmT[:], mirow4[:], id4, start=True, stop=True)
            nc.vector.tensor_copy(outrow[:, 5:9], psmT[:])
            nc.vector.tensor_reduce(outrow[:, 4:5], psmT[:], AX.X, OP.min)
            nc.vector.tensor_tensor(outrow[:, 0:1], outrow[:, 4:5], tanhd[:],
                                    OP.add)
            nc.sync.dma_start(out=out[:], in_=outrow[:])
            if debug:
                dbs = sb.tile([128, 16], F32, tag="dbs")
                nc.scalar.copy(dbs[:], psS[:, 0:16])
                nc.sync.dma_start(out=dbg_s[:], in_=dbs[:])
                nc.sync.dma_start(out=dbg_mm[:], in_=gmm[:])
                nc.sync.dma_start(out=dbg_bin[:], in_=binint[:, 0:16])
                nc.sync.dma_start(out=dbg_jnt[:], in_=jnt[:])
                nc.sync.dma_start(out=dbg_mi[:], in_=outrow[:])

    nc.compile()
    return nc


def mybir_reduce_max():
    import concourse.bass_isa as bass_isa
    return bass_isa.ReduceOp.max


def _get_nc(debug=False):
    key = ("ncd" if debug else "nc")
    if key not in _CACHE:
        _CACHE[key] = _build(debug)
    return _CACHE[key]


def kernel(state, state_memory, state_history, partitions, sample_idx,
           trace=False, debug=False):
    global LAST_RESULTS
    state = np.asarray(state, np.float32)
    state_memory = np.asarray(state_memory, np.float32)
    state_history = np.asarray(state_history, np.float32)
    partitions = np.asarray(partitions)
    sample_idx = np.asarray(sample_idx)

    pf = partitions.astype(np.float32)
    m8 = np.empty((D, J), np.float16)
    invc8 = np.empty((J,), np.float32)
    for p in range(NPAIR):
        m8[:, p] = pf[p]
        m8[:, NPAIR + p] = np.float32(1.0) - pf[p]
        invc8[p] = np.float32(1.0) / pf[p].sum(dtype=np.float32)
        invc8[NPAIR + p] = np.float32(1.0) / (np.float32(1.0) - pf[p]).sum(
            dtype=np.float32)
    cst1 = np.zeros((40, 20), np.float32)
    for p in range(NPAIR):
        cst1[NB * p:NB * (p + 1), p] = 1.0
    cst1[0, 4:12] = invc8
    cst1[0, 12:20] = np.float32(10.0) * invc8
    cst2 = np.zeros((4, 44), np.float32)
    for p in range(NPAIR):
        cst2[p, NB * p:NB * (p + 1)] = 1.0
        cst2[p, 40 + p] = 1.0

    memory = np.concatenate([state, state_memory[state.shape[0]:]], axis=0)
    memt = np.ascontiguousarray(memory.T).astype(np.float16)
    sampt = np.ascontiguousarray(memory[sample_idx].T).astype(np.float16)

    in_maps = []
    for c in range(N_CORES):
        htc = np.ascontiguousarray(
            state_history[c * TL:(c + 1) * TL, :].T).astype(np.float16)
        in_maps.append({"ht": htc, "m8": m8, "memt": memt, "sampt": sampt,
                        "cst1": cst1, "cst2": cst2})

    nc = _get_nc(debug)
    res = run_bass_kernel_spmd(nc, in_maps, list(range(N_CORES)),
                               trace=trace)
    LAST_RESULTS = res
    return np.asarray(res.results[0]["out"], np.float32)


# revision 18
# speedup vs baseline: 2.1877x; 1.0056x over previous
"""Trainium2 Bass kernel for nn_ConsciousnessMonitor (histogram_binning).

kernel(**inputs) takes FULL unsharded numpy inputs, returns the full (9,)
float32 output. Shards state_history along time across 8 NeuronCores.

Design: state_history streamed as fp16 (halves HBM traffic; empirically
max-rel-err from the cast is 6.9e-3 vs the 2e-2 gate). The masked-mean
matmul runs transposed (H chunk stationary, 8 mask columns moving) so the
projections land time-on-partitions in a single PSUM region [128, 32*8];
that removes the affine/transpose stage entirely. Min/max reductions split
across DVE and Pool, then a 64B AllReduce of (max,-min); binning is two
DVE ops + 10 one-hot compares (DVE/Pool split) + 32 packed 40x40 joint
matmuls; a 1.6KB AllReduce of the 4 joint histograms; the MI tail is
batched across all 4 pairs (pairs stacked along partitions).

Self-contained: shapes/sharding hardcoded; reads no sibling files.
"""
import numpy as np

import concourse.bacc as bacc
import concourse.tile as tile
import concourse.mybir as mybir
from concourse.bass_utils import run_bass_kernel_spmd

F32 = mybir.dt.float32
F16 = mybir.dt.float16
I16 = mybir.dt.int16
AX = mybir.AxisListType
OP = mybir.AluOpType
ACT = mybir.ActivationFunctionType

N_CORES = 8
T, D = 32768, 2048
TL = T // N_CORES          # 4096 time steps per core
NB = 10                    # histogram bins per axis
NPAIR = 4                  # partitions (mask pairs)
J = 2 * NPAIR              # 8 masked-sum series (x0..x3, y0..y3)
NDC = D // 128             # 16 contraction chunks
NCH = TL // 128            # 32 time chunks of 128 (partition-major)
MEM = 100
SN = 10

# DMA segments: (dk, t0, t1); last chunk split so its matmul tail is short
SEGS = [(dk, 0, TL) for dk in range(NDC - 1)]
SEGS += [(NDC - 1, 0, TL // 2), (NDC - 1, TL // 2, TL)]

_CACHE = {}
LAST_RESULTS = None


def _build(debug=False, variant="main"):
    sim1 = variant.startswith("sim1")
    nc = bacc.Bacc("TRN2", target_bir_lowering=False, debug=False,
                   num_devices=1 if sim1 else N_CORES)
    ht = nc.dram_tensor("ht", [D, TL], F16, kind="ExternalInput").ap()
    m8 = nc.dram_tensor("m8", [D, 2 * J], F16, kind="ExternalInput").ap()
    memt = nc.dram_tensor("memt", [D, MEM], F16, kind="ExternalInput").ap()
    sampt = nc.dram_tensor("sampt", [D, SN], F16, kind="ExternalInput").ap()
    # cst1 = blkones [40, 4] block indicator
    cst1 = nc.dram_tensor("cst1", [40, 4], F32, kind="ExternalInput").ap()
    # cst2: [4, 44] = blkT [4,40] | ident4 [4,4]
    cst2 = nc.dram_tensor("cst2", [4, 44], F32, kind="ExternalInput").ap()
    out = nc.dram_tensor("out", [9], F32, kind="ExternalOutput").ap()
    if debug:
        dbg_s = nc.dram_tensor("dbg_s", [128, 16], F32, kind="ExternalOutput").ap()
        dbg_mm = nc.dram_tensor("dbg_mm", [1, 16], F32, kind="ExternalOutput").ap()
        dbg_bin = nc.dram_tensor("dbg_bin", [128, 16], I16, kind="ExternalOutput").ap()
        dbg_jnt = nc.dram_tensor("dbg_jnt", [40, 10], F32, kind="ExternalOutput").ap()
        dbg_mi = nc.dram_tensor("dbg_mi", [1, 9], F32, kind="ExternalOutput").ap()

    rg = [list(range(N_CORES))]

    with tile.TileContext(nc) as tc:
        with tc.tile_pool(name="consts", bufs=1) as consts, \
             tc.tile_pool(name="sb", bufs=1) as sb, \
             tc.tile_pool(name="htp", bufs=4) as htp, \
             tc.tile_pool(name="psS", bufs=1, space="PSUM") as psS_pool, \
             tc.tile_pool(name="psJ", bufs=4, space="PSUM") as psJ_pool, \
             tc.tile_pool(name="misc", bufs=3, space="PSUM") as misc, \
             tc.tile_pool(name="dram", bufs=1, space="DRAM") as dram:

            # ---- mask matrix first: gates the stream matmuls ----
            m_sb = consts.tile([128, NDC * 2 * J], F16, tag="msb")
            nc.scalar.dma_start(
                out=m_sb[:].rearrange("p (k j) -> p k j", j=2 * J),
                in_=m8.rearrange("(k p) j -> p k j", p=128))
            ones128 = consts.tile([128, 1], F32, tag="o128")
            nc.gpsimd.memset(ones128[:], 1.0)
            ones10 = consts.tile([SN, 1], F32, tag="o10")
            nc.gpsimd.memset(ones10[:], 1.0)
            ones1_10 = consts.tile([1, SN], F32, tag="o110")
            nc.vector.memset(ones1_10[:], 1.0)

            # ---- stream: psS[t128, (tc, j)] += ht_chunk.T @ m ----
            # moving operand = [masks | -masks]: psS[:, :, 8:16] = -S, so
            # one X-reduce max yields (max | -min) directly
            psS = psS_pool.tile([128, NCH * 2 * J], F32, tag="psS")
            for si, (dk, t0, t1) in enumerate(SEGS):
                htt = htp.tile([128, t1 - t0], F16, tag="htt", name="htt")
                nc.sync.dma_start(out=htt[:], in_=ht[dk * 128:(dk + 1) * 128,
                                                     t0:t1])
                for tcn in range(t0 // 128, t1 // 128):
                    # start=True zeroes the whole 2KB PSUM region, so only
                    # the very first matmul starts; only the last stops
                    nc.tensor.matmul(
                        psS[:, tcn * 2 * J:(tcn + 1) * 2 * J],
                        htt[:, tcn * 128 - t0:(tcn + 1) * 128 - t0],
                        m_sb[:, dk * 2 * J:(dk + 1) * 2 * J],
                        start=(si == 0 and tcn == 0),
                        stop=(si == len(SEGS) - 1 and tcn == NCH - 1))

            # ---- small inputs: issued late so their DMA slots fall after
            # the stream on the shared DMA engines; consumers have slack ----
            cst1_sb = consts.tile([40, 4], F32, tag="cst1")
            nc.scalar.dma_start(out=cst1_sb[:], in_=cst1[:])
            cst2_sb = consts.tile([4, 44], F32, tag="cst2")
            nc.scalar.dma_start(out=cst2_sb[:], in_=cst2[:])
            blkA = cst1_sb[:]               # [40,4] block indicator
            blkT = cst2_sb[:, 0:40]         # [4,40]
            id4 = cst2_sb[:, 40:44]         # [4,4]
            mem_sb = consts.tile([128, NDC * MEM], F16, tag="memsb")
            nc.scalar.dma_start(
                out=mem_sb[:].rearrange("p (k f) -> p k f", f=MEM),
                in_=memt.rearrange("(k p) f -> p k f", p=128))
            samp_sb = consts.tile([128, NDC * SN], F16, tag="sampsb")
            nc.scalar.dma_start(
                out=samp_sb[:].rearrange("p (k f) -> p k f", f=SN),
                in_=sampt.rearrange("(k p) f -> p k f", p=128))

            # ---- (max | -min) in one reduce, partition-reduce, AllReduce --
            pboth = sb.tile([128, 2 * J], F32, tag="pboth")
            nc.vector.tensor_reduce(
                pboth[:], psS[:].rearrange("p (c g) -> p g c", g=2 * J),
                AX.X, OP.max)
            arb = sb.tile([128, 2 * J], F32, tag="arb")
            nc.gpsimd.partition_all_reduce(arb[:], pboth[:], 128,
                                           mybir_reduce_max())
            cbA = dram.tile([1, 2 * J], F32, tag="cba")
            cbB = dram.tile([1, 2 * J], F32, tag="cbb")
            nc.sync.dma_start(out=cbA[:], in_=arb[0:1, :])
            gmm = sb.tile([1, 2 * J], F32, tag="gmm")
            if sim1:
                nc.scalar.dma_start(out=gmm[:], in_=cbA[:])
            else:
                nc.gpsimd.collective_compute("AllReduce", OP.max,
                                             replica_groups=rg,
                                             ins=[cbA.opt()],
                                             outs=[cbB.opt()])
                nc.scalar.dma_start(out=gmm[:], in_=cbB[:])

            # ---- differentiation branch (fills the AllReduce wait) ----
            outrow = sb.tile([1, 9], F32, tag="outrow")
            psG = misc.tile([SN, SN], F32, tag="m", name="psG")
            for k in range(NDC):
                nc.tensor.matmul(psG[:], samp_sb[:, k * SN:(k + 1) * SN],
                                 samp_sb[:, k * SN:(k + 1) * SN],
                                 start=(k == 0), stop=(k == NDC - 1))
            sqs = sb.tile([128, NDC * SN], F32, tag="sqs")
            nc.vector.tensor_tensor(sqs[:], samp_sb[:], samp_sb[:], OP.mult)
            psrc = misc.tile([SN, 1], F32, tag="m", name="psrc")
            psrr = misc.tile([1, SN], F32, tag="m", name="psrr")
            for k in range(NDC):
                nc.tensor.matmul(psrc[:], sqs[:, k * SN:(k + 1) * SN],
                                 ones128[:], start=(k == 0),
                                 stop=(k == NDC - 1))
                nc.tensor.matmul(psrr[:], ones128[:],
                                 sqs[:, k * SN:(k + 1) * SN], start=(k == 0),
                                 stop=(k == NDC - 1))
            g_sb = sb.tile([SN, SN], F32, tag="gsb")
            nc.scalar.copy(g_sb[:], psG[:])
            rcol = sb.tile([SN, 1], F32, tag="rcol")
            nc.scalar.copy(rcol[:], psrc[:])
            rrow = sb.tile([1, SN], F32, tag="rrow")
            nc.scalar.copy(rrow[:], psrr[:])

            mem3 = mem_sb[:].rearrange("p (k f) -> p k f", f=MEM)
            mean16 = sb.tile([128, NDC], F32, tag="mean16")
            nc.vector.tensor_reduce(mean16[:], mem3, AX.X, OP.add)
            nc.vector.tensor_scalar(mean16[:], mean16[:], 1.0 / MEM, None,
                                    OP.mult)
            cent = sb.tile([128, NDC * MEM], F32, tag="cent")
            nc.vector.tensor_tensor(
                cent[:].rearrange("p (k f) -> p k f", f=MEM), mem3,
                mean16[:, :, None].broadcast_to([128, NDC, MEM]), OP.subtract)
            nc.vector.tensor_tensor(cent[:], cent[:], cent[:], OP.mult)
            var16 = sb.tile([128, NDC], F32, tag="var16")
            nc.vector.tensor_reduce(
                var16[:], cent[:].rearrange("p (k f) -> p k f", f=MEM),
                AX.X, OP.add)
            nc.vector.tensor_scalar(var16[:], var16[:], 1.0 / (MEM - 1), None,
                                    OP.mult)
            redv = sb.tile([128, 1], F32, tag="redv")
            nc.vector.tensor_reduce(redv[:], var16[:], AX.X, OP.add)
            v2 = sb.tile([128, NDC], F32, tag="v2")
            nc.vector.tensor_tensor(v2[:], var16[:], var16[:], OP.mult)
            redv2 = sb.tile([128, 1], F32, tag="redv2")
            nc.vector.tensor_reduce(redv2[:], v2[:], AX.X, OP.add)
            pstv = misc.tile([1, 1], F32, tag="m", name="pstv")
            nc.tensor.matmul(pstv[:], redv[:], ones128[:], start=True,
                             stop=True)
            tv_sb = outrow[:, 3:4]
            nc.scalar.copy(tv_sb, pstv[:])
            pss2 = misc.tile([1, 1], F32, tag="m", name="pss2")
            nc.tensor.matmul(pss2[:], redv2[:], ones128[:], start=True,
                             stop=True)
            s2_sb = sb.tile([1, 1], F32, tag="s2sb")
            nc.scalar.copy(s2_sb[:], pss2[:])

            tvsq = sb.tile([1, 1], F32, tag="tvsq")
            nc.vector.tensor_tensor(tvsq[:], tv_sb, tv_sb, OP.mult)
            dden = sb.tile([1, 1], F32, tag="dden")
            nc.vector.scalar_tensor_tensor(dden[:], tvsq[:], 1e-6, s2_sb[:],
                                           OP.mult, OP.add)
            rdden = sb.tile([1, 1], F32, tag="rdden")
            nc.vector.reciprocal(rdden[:], dden[:])
            nc.vector.tensor_tensor(outrow[:, 2:3], tvsq[:], rdden[:],
                                    OP.mult)

            # cdist tail: d2 = r_i + r_j - 2G
            rB = misc.tile([SN, SN], F32, tag="m", name="rB")
            nc.tensor.matmul(rB[:], ones1_10[:], rrow[:], start=True,
                             stop=True)
            d2 = sb.tile([SN, SN], F32, tag="d2")
            nc.vector.scalar_tensor_tensor(d2[:], g_sb[:], -2.0, rB[:],
                                           OP.mult, OP.add)
            nc.vector.tensor_scalar(d2[:], d2[:], rcol[:], 0.0, OP.add,
                                    OP.max)
            dst = sb.tile([SN, SN], F32, tag="dst")
            nc.scalar.activation(dst[:], d2[:], ACT.Sqrt)
            dsum = sb.tile([SN, 1], F32, tag="dsum")
            nc.vector.tensor_reduce(dsum[:], dst[:], AX.X, OP.add)
            psD = misc.tile([1, 1], F32, tag="m", name="psD")
            nc.tensor.matmul(psD[:], dsum[:], ones10[:], start=True, stop=True)
            avg_sb = sb.tile([1, 1], F32, tag="avgsb")
            nc.vector.tensor_scalar(avg_sb[:], psD[:],
                                    float(1.0 / (SN * (SN - 1) + 1e-6)), None,
                                    OP.mult)
            sqtv = sb.tile([1, 1], F32, tag="sqtv")
            nc.scalar.activation(sqtv[:], tv_sb, ACT.Sqrt)
            diff_sb = outrow[:, 1:2]
            nc.vector.tensor_tensor(diff_sb, sqtv[:], avg_sb[:], OP.mult)
            tanhd = sb.tile([1, 1], F32, tag="tanhd")
            nc.scalar.activation(tanhd[:], diff_sb, ACT.Tanh)
            # preload the Ln table during collective slack so the MI-tail
            # Ln doesn't pay the 1.3us table switch (copies stay in-set)
            lnwarm = sb.tile([1, 1], F32, tag="lnwarm")
            nc.scalar.activation(lnwarm[:], tanhd[:], ACT.Ln)

            # ---- bin coefficients from global (vmax | -vmin); masks are
            # pre-scaled by 1/count so psS holds mean-scale values ----
            # s = 10/(vmax-vmin+1e-6); s1 = s; b1 = -vmin*s - 0.5
            den = sb.tile([1, J], F32, tag="den")
            nc.vector.tensor_tensor(den[:], gmm[:, 0:J], gmm[:, J:2 * J],
                                    OP.add)
            nc.vector.tensor_scalar(den[:], den[:], 1e-6, None, OP.add)
            rden = sb.tile([1, J], F32, tag="rden")
            nc.vector.reciprocal(rden[:], den[:])
            coefr = sb.tile([1, 2 * J], F32, tag="coefr")
            nc.vector.tensor_scalar(coefr[:, 0:J], rden[:], 10.0, None,
                                    OP.mult)
            t1 = sb.tile([1, J], F32, tag="t1")
            nc.vector.tensor_tensor(t1[:], gmm[:, J:2 * J], rden[:], OP.mult)
            nc.vector.tensor_scalar(coefr[:, J:2 * J], t1[:], 10.0, -0.5,
                                    OP.mult, OP.add)
            coef = sb.tile([128, 2 * J], F32, tag="coef")
            nc.gpsimd.partition_broadcast(coef[:], coefr[:])

            # ---- binning: binint = RNE(psS*s1 + b1) as int16 ----
            binf = sb.tile([128, NCH * J], F32, tag="binf")
            bf3 = binf[:].rearrange("p (c j) -> p c j", j=J)
            nc.vector.tensor_tensor(
                bf3,
                psS[:].rearrange("p (c j) -> p c j", j=2 * J)[:, :, 0:J],
                coef[:, None, 0:J].broadcast_to([128, NCH, J]), OP.mult)
            binint = sb.tile([128, NCH * J], I16, tag="binint")
            nc.vector.tensor_tensor(
                binint[:].rearrange("p (c j) -> p c j", j=J), bf3,
                coef[:, None, J:2 * J].broadcast_to([128, NCH, J]), OP.add)

            # ---- one-hot (7 bins on DVE, 3 on Pool; edge bins clamp) ----
            ohsb = sb.tile([128, NCH * J * NB], F16, tag="ohsb")
            oh4 = ohsb[:].rearrange("p (c j b) -> p c j b", j=J, b=NB)
            bi3 = binint[:].rearrange("p (c j) -> p c j", j=J)
            for b in range(NB):
                eng = nc.vector if b < 7 else nc.gpsimd
                op = OP.is_le if b == 0 else (OP.is_ge if b == NB - 1
                                              else OP.is_equal)
                eng.tensor_scalar(oh4[:, :, :, b], bi3, b, None, op)

            # ---- joint histograms: pair p -> its own PSUM bank tile ----
            psJp = [psJ_pool.tile([NB, NB], F32, tag="psJ", name=f"psJ{p}")
                    for p in range(NPAIR)]
            for c in range(NCH):
                base = c * J * NB
                for p in range(NPAIR):
                    nc.tensor.matmul(
                        psJp[p][:],
                        ohsb[:, base + NB * p:base + NB * (p + 1)],
                        ohsb[:, base + 40 + NB * p:base + 40 + NB * (p + 1)],
                        start=(c == 0), stop=(c == NCH - 1))
            jm = sb.tile([NB, NPAIR * NB], F32, tag="jm")
            nc.scalar.copy(jm[:, 0:NB], psJp[0][:])
            nc.vector.tensor_copy(jm[:, NB:2 * NB], psJp[1][:])
            nc.scalar.copy(jm[:, 2 * NB:3 * NB], psJp[2][:])
            nc.vector.tensor_copy(jm[:, 3 * NB:4 * NB], psJp[3][:])
            cbj = dram.tile([NPAIR, NB * NB], F32, tag="cbj")
            cbj2 = dram.tile([NPAIR, NB * NB], F32, tag="cbj2")
            nc.sync.dma_start(
                out=cbj[:].rearrange("p (a b) -> a p b", a=NB),
                in_=jm[:].rearrange("a (p b) -> a p b", b=NB))
            jnt = sb.tile([40, NB], F32, tag="jnt")
            if sim1:
                nc.sync.dma_start(
                    out=jnt[:], in_=cbj.rearrange("p (a b) -> (p a) b", a=NB))
            else:
                nc.gpsimd.collective_compute("AllReduce", OP.add,
                                             replica_groups=rg,
                                             ins=[cbj.opt()],
                                             outs=[cbj2.opt()])
                nc.sync.dma_start(
                    out=jnt[:], in_=cbj2.rearrange("p (a b) -> (p a) b", a=NB))

            # ---- MI, batched across pairs (pairs along partitions) ----
            rowsum = sb.tile([40, 1], F32, tag="rowsum")
            nc.vector.tensor_reduce(rowsum[:], jnt[:], AX.X, OP.add)
            pscol = misc.tile([NPAIR, NB], F32, tag="m", name="pscol")
            nc.tensor.matmul(pscol[:], blkA, jnt[:], start=True, stop=True)
            tot4 = sb.tile([NPAIR, 1], F32, tag="tot4")
            nc.vector.tensor_reduce(tot4[:], pscol[:], AX.X, OP.add)
            tinv4 = sb.tile([NPAIR, 1], F32, tag="tinv4")
            nc.vector.reciprocal(tinv4[:], tot4[:])
            pst40 = misc.tile([40, 1], F32, tag="m", name="pst40")
            nc.tensor.matmul(pst40[:], blkT, tinv4[:], start=True, stop=True)
            tinv40 = sb.tile([40, 1], F32, tag="tinv40")
            nc.vector.tensor_copy(tinv40[:], pst40[:])
            px = sb.tile([40, 1], F32, tag="px")
            nc.vector.tensor_tensor(px[:], rowsum[:], tinv40[:], OP.mult)
            py4 = sb.tile([NPAIR, NB], F32, tag="py4")
            nc.vector.tensor_scalar(py4[:], pscol[:], tinv4[:], None, OP.mult)
            pspy = misc.tile([40, NB], F32, tag="m", name="pspy")
            nc.tensor.matmul(pspy[:], blkT, py4[:], start=True, stop=True)
            jn = sb.tile([40, NB], F32, tag="jn")
            nc.vector.tensor_scalar(jn[:], jnt[:], tinv40[:], None, OP.mult)
            outer = sb.tile([40, NB], F32, tag="outer")
            nc.vector.tensor_scalar(outer[:], pspy[:], px[:], 1e-10, OP.mult,
                                    OP.add)
            rout = sb.tile([40, NB], F32, tag="rout")
            nc.vector.reciprocal(rout[:], outer[:])
            num = sb.tile([40, NB], F32, tag="num")
            nc.vector.scalar_tensor_tensor(num[:], jn[:], 1e-10, rout[:],
                                           OP.add, OP.mult)
            lg = sb.tile([40, NB], F32, tag="lg")
            nc.scalar.activation(lg[:], num[:], ACT.Ln)
            nc.vector.tensor_tensor(lg[:], jn[:], lg[:], OP.mult)
            ms = sb.tile([40, 1], F32, tag="ms")
            nc.vector.tensor_reduce(ms[:], lg[:], AX.X, OP.add)
            psmi = misc.tile([NPAIR, 1], F32, tag="m", name="psmi")
            nc.tensor.matmul(psmi[:], blkA, ms[:], start=True, stop=True)
            mirow4 = sb.tile([NPAIR, 1], F32, tag="mirow4")
            nc.vector.tensor_scalar(mirow4[:], psmi[:], 0.0, None, OP.max)
            psmT = misc.tile([1, NPAIR], F32, tag="m", name="psmT")
            nc.tensor.matmul(psmT[:], mirow4[:], id4, start=True, stop=True)
            nc.vector.tensor_copy(outrow[:, 5:9], psmT[:])
            nc.vector.tensor_reduce(outrow[:, 4:5], psmT[:], AX.X, OP.min)
            nc.vector.tensor_tensor(outrow[:, 0:1], outrow[:, 4:5], tanhd[:],
                                    OP.add)
            nc.sync.dma_start(out=out[:], in_=outrow[:])
            if debug:
                dbs = sb.tile([128, 16], F32, tag="dbs")
                nc.scalar.copy(dbs[:], psS[:, 0:16])
                nc.sync.dma_start(out=dbg_s[:], in_=dbs[:])
                nc.sync.dma_start(out=dbg_mm[:], in_=gmm[:])
                nc.sync.dma_start(out=dbg_bin[:], in_=binint[:, 0:16])
                nc.sync.dma_start(out=dbg_jnt[:], in_=jnt[:])
                nc.sync.dma_start(out=dbg_mi[:], in_=outrow[:])

    nc.compile()
    return nc


def mybir_reduce_max():
    import concourse.bass_isa as bass_isa
    return bass_isa.ReduceOp.max


def _get_nc(debug=False):
    key = ("ncd" if debug else "nc")
    if key not in _CACHE:
        _CACHE[key] = _build(debug)
    return _CACHE[key]


def kernel(state, state_memory, state_history, partitions, sample_idx,
           trace=False, debug=False):
    global LAST_RESULTS
    state = np.asarray(state, np.float32)
    state_memory = np.asarray(state_memory, np.float32)
    state_history = np.asarray(state_history, np.float32)
    partitions = np.asarray(partitions)
    sample_idx = np.asarray(sample_idx)

    pf = partitions.astype(np.float32)
    m8 = np.empty((D, 2 * J), np.float16)
    invc8 = np.empty((J,), np.float32)
    for p in range(NPAIR):
        m8[:, p] = pf[p]
        m8[:, NPAIR + p] = np.float32(1.0) - pf[p]
        invc8[p] = np.float32(1.0) / pf[p].sum(dtype=np.float32)
        invc8[NPAIR + p] = np.float32(1.0) / (np.float32(1.0) - pf[p]).sum(
            dtype=np.float32)
    m8[:, 0:J] = (m8[:, 0:J].astype(np.float32) * invc8[None, :]).astype(
        np.float16)
    m8[:, J:2 * J] = -m8[:, 0:J]
    cst1 = np.zeros((40, 4), np.float32)
    for p in range(NPAIR):
        cst1[NB * p:NB * (p + 1), p] = 1.0
    cst2 = np.zeros((4, 44), np.float32)
    for p in range(NPAIR):
        cst2[p, NB * p:NB * (p + 1)] = 1.0
        cst2[p, 40 + p] = 1.0

    memory = np.concatenate([state, state_memory[state.shape[0]:]], axis=0)
    memt = np.ascontiguousarray(memory.T).astype(np.float16)
    sampt = np.ascontiguousarray(memory[sample_idx].T).astype(np.float16)

    in_maps = []
    for c in range(N_CORES):
        htc = np.ascontiguousarray(
            state_history[c * TL:(c + 1) * TL, :].T).astype(np.float16)
        in_maps.append({"ht": htc, "m8": m8, "memt": memt, "sampt": sampt,
                        "cst1": cst1, "cst2": cst2})

    nc = _get_nc(debug)
    res = run_bass_kernel_spmd(nc, in_maps, list(range(N_CORES)),
                               trace=trace)
    LAST_RESULTS = res
    return np.asarray(res.results[0]["out"], np.float32)


# revision 19
# speedup vs baseline: 2.1971x; 1.0043x over previous
"""Trainium2 Bass kernel for nn_ConsciousnessMonitor (histogram_binning).

kernel(**inputs) takes FULL unsharded numpy inputs, returns the full (9,)
float32 output. Shards state_history along time across 8 NeuronCores.

Design: state_history streamed as fp16 (halves HBM traffic; empirically
max-rel-err from the cast is 6.9e-3 vs the 2e-2 gate). The masked-mean
matmul runs transposed (H chunk stationary, 8 mask columns moving) so the
projections land time-on-partitions in a single PSUM region [128, 32*8];
that removes the affine/transpose stage entirely. Min/max reductions split
across DVE and Pool, then a 64B AllReduce of (max,-min); binning is two
DVE ops + 10 one-hot compares (DVE/Pool split) + 32 packed 40x40 joint
matmuls; a 1.6KB AllReduce of the 4 joint histograms; the MI tail is
batched across all 4 pairs (pairs stacked along partitions).

Self-contained: shapes/sharding hardcoded; reads no sibling files.
"""
import numpy as np

import concourse.bacc as bacc
import concourse.tile as tile
import concourse.mybir as mybir
from concourse.bass_utils import run_bass_kernel_spmd

F32 = mybir.dt.float32
F16 = mybir.dt.float16
I16 = mybir.dt.int16
AX = mybir.AxisListType
OP = mybir.AluOpType
ACT = mybir.ActivationFunctionType

N_CORES = 8
T, D = 32768, 2048
TL = T // N_CORES          # 4096 time steps per core
NB = 10                    # histogram bins per axis
NPAIR = 4                  # partitions (mask pairs)
J = 2 * NPAIR              # 8 masked-sum series (x0..x3, y0..y3)
NDC = D // 128             # 16 contraction chunks
NCH = TL // 128            # 32 time chunks of 128 (partition-major)
MEM = 100
SN = 10

# DMA segments: (dk, t0, t1); last chunk split so its matmul tail is short
SEGS = [(dk, 0, TL) for dk in range(NDC - 1)]
SEGS += [(NDC - 1, 0, TL // 2), (NDC - 1, TL // 2, TL)]

_CACHE = {}
LAST_RESULTS = None


def _build(debug=False, variant="main"):
    sim1 = variant.startswith("sim1")
    nc = bacc.Bacc("TRN2", target_bir_lowering=False, debug=False,
                   num_devices=1 if sim1 else N_CORES)
    ht = nc.dram_tensor("ht", [D, TL], F16, kind="ExternalInput").ap()
    m8 = nc.dram_tensor("m8", [D, 2 * J], F16, kind="ExternalInput").ap()
    memt = nc.dram_tensor("memt", [D, MEM], F16, kind="ExternalInput").ap()
    sampt = nc.dram_tensor("sampt", [D, SN], F16, kind="ExternalInput").ap()
    # cst1 = blkones [40, 4] block indicator
    cst1 = nc.dram_tensor("cst1", [40, 4], F32, kind="ExternalInput").ap()
    # cst2: [4, 44] = blkT [4,40] | ident4 [4,4]
    cst2 = nc.dram_tensor("cst2", [4, 44], F32, kind="ExternalInput").ap()
    out = nc.dram_tensor("out", [9], F32, kind="ExternalOutput").ap()
    if debug:
        dbg_s = nc.dram_tensor("dbg_s", [128, 16], F32, kind="ExternalOutput").ap()
        dbg_mm = nc.dram_tensor("dbg_mm", [1, 16], F32, kind="ExternalOutput").ap()
        dbg_bin = nc.dram_tensor("dbg_bin", [128, 16], I16, kind="ExternalOutput").ap()
        dbg_jnt = nc.dram_tensor("dbg_jnt", [40, 10], F32, kind="ExternalOutput").ap()
        dbg_mi = nc.dram_tensor("dbg_mi", [1, 9], F32, kind="ExternalOutput").ap()

    rg = [list(range(N_CORES))]

    with tile.TileContext(nc) as tc:
        with tc.tile_pool(name="consts", bufs=1) as consts, \
             tc.tile_pool(name="sb", bufs=1) as sb, \
             tc.tile_pool(name="htp", bufs=4) as htp, \
             tc.tile_pool(name="psS", bufs=1, space="PSUM") as psS_pool, \
             tc.tile_pool(name="psJ", bufs=4, space="PSUM") as psJ_pool, \
             tc.tile_pool(name="misc", bufs=3, space="PSUM") as misc, \
             tc.tile_pool(name="dram", bufs=1, space="DRAM") as dram:

            # ---- mask matrix first: gates the stream matmuls ----
            m_sb = consts.tile([128, NDC * 2 * J], F16, tag="msb")
            nc.scalar.dma_start(
                out=m_sb[:].rearrange("p (k j) -> p k j", j=2 * J),
                in_=m8.rearrange("(k p) j -> p k j", p=128))
            ones128 = consts.tile([128, 1], F32, tag="o128")
            nc.gpsimd.memset(ones128[:], 1.0)
            ones10 = consts.tile([SN, 1], F32, tag="o10")
            nc.gpsimd.memset(ones10[:], 1.0)
            ones1_10 = consts.tile([1, SN], F32, tag="o110")
            nc.vector.memset(ones1_10[:], 1.0)

            # ---- stream: psS[t128, (tc, j)] += ht_chunk.T @ m ----
            # moving operand = [masks | -masks]: psS[:, :, 8:16] = -S, so
            # one X-reduce max yields (max | -min) directly
            psS = psS_pool.tile([128, NCH * 2 * J], F32, tag="psS")
            for si, (dk, t0, t1) in enumerate(SEGS):
                htt = htp.tile([128, t1 - t0], F16, tag="htt", name="htt")
                nc.sync.dma_start(out=htt[:], in_=ht[dk * 128:(dk + 1) * 128,
                                                     t0:t1])
                for tcn in range(t0 // 128, t1 // 128):
                    # start=True zeroes the whole 2KB PSUM region, so only
                    # the very first matmul starts; only the last stops
                    nc.tensor.matmul(
                        psS[:, tcn * 2 * J:(tcn + 1) * 2 * J],
                        htt[:, tcn * 128 - t0:(tcn + 1) * 128 - t0],
                        m_sb[:, dk * 2 * J:(dk + 1) * 2 * J],
                        start=(si == 0 and tcn == 0),
                        stop=(si == len(SEGS) - 1 and tcn == NCH - 1))

            # ---- small inputs: issued late so their DMA slots fall after
            # the stream on the shared DMA engines; consumers have slack ----
            cst1_sb = consts.tile([40, 4], F32, tag="cst1")
            nc.scalar.dma_start(out=cst1_sb[:], in_=cst1[:])
            cst2_sb = consts.tile([4, 44], F32, tag="cst2")
            nc.scalar.dma_start(out=cst2_sb[:], in_=cst2[:])
            blkA = cst1_sb[:]               # [40,4] block indicator
            blkT = cst2_sb[:, 0:40]         # [4,40]
            id4 = cst2_sb[:, 40:44]         # [4,4]
            mem_sb = consts.tile([128, NDC * MEM], F16, tag="memsb")
            nc.scalar.dma_start(
                out=mem_sb[:].rearrange("p (k f) -> p k f", f=MEM),
                in_=memt.rearrange("(k p) f -> p k f", p=128))
            samp_sb = consts.tile([128, NDC * SN], F16, tag="sampsb")
            nc.scalar.dma_start(
                out=samp_sb[:].rearrange("p (k f) -> p k f", f=SN),
                in_=sampt.rearrange("(k p) f -> p k f", p=128))

            # ---- (max | -min) in one reduce, partition-reduce, AllReduce --
            pboth = sb.tile([128, 2 * J], F32, tag="pboth")
            nc.vector.tensor_reduce(
                pboth[:], psS[:].rearrange("p (c g) -> p g c", g=2 * J),
                AX.X, OP.max)
            arb = sb.tile([128, 2 * J], F32, tag="arb")
            nc.gpsimd.partition_all_reduce(arb[:], pboth[:], 128,
                                           mybir_reduce_max())
            cbA = dram.tile([1, 2 * J], F32, tag="cba")
            cbB = dram.tile([1, 2 * J], F32, tag="cbb")
            nc.sync.dma_start(out=cbA[:], in_=arb[0:1, :])
            gmm = sb.tile([1, 2 * J], F32, tag="gmm")
            if sim1:
                nc.scalar.dma_start(out=gmm[:], in_=cbA[:])
            else:
                nc.gpsimd.collective_compute("AllReduce", OP.max,
                                             replica_groups=rg,
                                             ins=[cbA.opt()],
                                             outs=[cbB.opt()])
                nc.scalar.dma_start(out=gmm[:], in_=cbB[:])

            # ---- differentiation branch (fills the AllReduce wait) ----
            outrow = sb.tile([1, 9], F32, tag="outrow")
            psG = misc.tile([SN, SN], F32, tag="m", name="psG")
            for k in range(NDC):
                nc.tensor.matmul(psG[:], samp_sb[:, k * SN:(k + 1) * SN],
                                 samp_sb[:, k * SN:(k + 1) * SN],
                                 start=(k == 0), stop=(k == NDC - 1))
            sqs = sb.tile([128, NDC * SN], F32, tag="sqs")
            nc.vector.tensor_tensor(sqs[:], samp_sb[:], samp_sb[:], OP.mult)
            psrc = misc.tile([SN, 1], F32, tag="m", name="psrc")
            psrr = misc.tile([1, SN], F32, tag="m", name="psrr")
            for k in range(NDC):
                nc.tensor.matmul(psrc[:], sqs[:, k * SN:(k + 1) * SN],
                                 ones128[:], start=(k == 0),
                                 stop=(k == NDC - 1))
                nc.tensor.matmul(psrr[:], ones128[:],
                                 sqs[:, k * SN:(k + 1) * SN], start=(k == 0),
                                 stop=(k == NDC - 1))
            g_sb = sb.tile([SN, SN], F32, tag="gsb")
            nc.scalar.copy(g_sb[:], psG[:])
            rcol = sb.tile([SN, 1], F32, tag="rcol")
            nc.scalar.copy(rcol[:], psrc[:])
            rrow = sb.tile([1, SN], F32, tag="rrow")
            nc.scalar.copy(rrow[:], psrr[:])

            mem3 = mem_sb[:].rearrange("p (k f) -> p k f", f=MEM)
            mean16 = sb.tile([128, NDC], F32, tag="mean16")
            nc.vector.tensor_reduce(mean16[:], mem3, AX.X, OP.add)
            nc.vector.tensor_scalar(mean16[:], mean16[:], 1.0 / MEM, None,
                                    OP.mult)
            cent = sb.tile([128, NDC * MEM], F32, tag="cent")
            nc.vector.tensor_tensor(
                cent[:].rearrange("p (k f) -> p k f", f=MEM), mem3,
                mean16[:, :, None].broadcast_to([128, NDC, MEM]), OP.subtract)
            nc.vector.tensor_tensor(cent[:], cent[:], cent[:], OP.mult)
            var16 = sb.tile([128, NDC], F32, tag="var16")
            nc.vector.tensor_reduce(
                var16[:], cent[:].rearrange("p (k f) -> p k f", f=MEM),
                AX.X, OP.add)
            nc.vector.tensor_scalar(var16[:], var16[:], 1.0 / (MEM - 1), None,
                                    OP.mult)
            redv = sb.tile([128, 1], F32, tag="redv")
            nc.vector.tensor_reduce(redv[:], var16[:], AX.X, OP.add)
            v2 = sb.tile([128, NDC], F32, tag="v2")
            nc.vector.tensor_tensor(v2[:], var16[:], var16[:], OP.mult)
            redv2 = sb.tile([128, 1], F32, tag="redv2")
            nc.vector.tensor_reduce(redv2[:], v2[:], AX.X, OP.add)
            pstv = misc.tile([1, 1], F32, tag="m", name="pstv")
            nc.tensor.matmul(pstv[:], redv[:], ones128[:], start=True,
                             stop=True)
            tv_sb = outrow[:, 3:4]
            nc.scalar.copy(tv_sb, pstv[:])
            pss2 = misc.tile([1, 1], F32, tag="m", name="pss2")
            nc.tensor.matmul(pss2[:], redv2[:], ones128[:], start=True,
                             stop=True)
            s2_sb = sb.tile([1, 1], F32, tag="s2sb")
            nc.scalar.copy(s2_sb[:], pss2[:])

            tvsq = sb.tile([1, 1], F32, tag="tvsq")
            nc.vector.tensor_tensor(tvsq[:], tv_sb, tv_sb, OP.mult)
            dden = sb.tile([1, 1], F32, tag="dden")
            nc.vector.scalar_tensor_tensor(dden[:], tvsq[:], 1e-6, s2_sb[:],
                                           OP.mult, OP.add)
            rdden = sb.tile([1, 1], F32, tag="rdden")
            nc.vector.reciprocal(rdden[:], dden[:])
            nc.vector.tensor_tensor(outrow[:, 2:3], tvsq[:], rdden[:],
                                    OP.mult)

            # cdist tail: d2 = r_i + r_j - 2G
            rB = misc.tile([SN, SN], F32, tag="m", name="rB")
            nc.tensor.matmul(rB[:], ones1_10[:], rrow[:], start=True,
                             stop=True)
            d2 = sb.tile([SN, SN], F32, tag="d2")
            nc.vector.scalar_tensor_tensor(d2[:], g_sb[:], -2.0, rB[:],
                                           OP.mult, OP.add)
            nc.vector.tensor_scalar(d2[:], d2[:], rcol[:], 0.0, OP.add,
                                    OP.max)
            dst = sb.tile([SN, SN], F32, tag="dst")
            nc.scalar.activation(dst[:], d2[:], ACT.Sqrt)
            dsum = sb.tile([SN, 1], F32, tag="dsum")
            nc.vector.tensor_reduce(dsum[:], dst[:], AX.X, OP.add)
            psD = misc.tile([1, 1], F32, tag="m", name="psD")
            nc.tensor.matmul(psD[:], dsum[:], ones10[:], start=True, stop=True)
            avg_sb = sb.tile([1, 1], F32, tag="avgsb")
            nc.vector.tensor_scalar(avg_sb[:], psD[:],
                                    float(1.0 / (SN * (SN - 1) + 1e-6)), None,
                                    OP.mult)
            sqtv = sb.tile([1, 1], F32, tag="sqtv")
            nc.scalar.activation(sqtv[:], tv_sb, ACT.Sqrt)
            diff_sb = outrow[:, 1:2]
            nc.vector.tensor_tensor(diff_sb, sqtv[:], avg_sb[:], OP.mult)
            tanhd = sb.tile([1, 1], F32, tag="tanhd")
            nc.scalar.activation(tanhd[:], diff_sb, ACT.Tanh)
            # preload the Ln table during collective slack so the MI-tail
            # Ln doesn't pay the 1.3us table switch (copies stay in-set)
            lnwarm = sb.tile([1, 1], F32, tag="lnwarm")
            nc.scalar.activation(lnwarm[:], tanhd[:], ACT.Ln)

            # ---- bin coefficients from global (vmax | -vmin); masks are
            # pre-scaled by 1/count so psS holds mean-scale values ----
            # s = 10/(vmax-vmin+1e-6); s1 = s; b1 = -vmin*s - 0.5
            den = sb.tile([1, J], F32, tag="den")
            nc.vector.tensor_tensor(den[:], gmm[:, 0:J], gmm[:, J:2 * J],
                                    OP.add)
            nc.vector.tensor_scalar(den[:], den[:], 1e-6, None, OP.add)
            rden = sb.tile([1, J], F32, tag="rden")
            nc.vector.reciprocal(rden[:], den[:])
            coefr = sb.tile([1, 2 * J], F32, tag="coefr")
            nc.vector.tensor_scalar(coefr[:, 0:J], rden[:], 10.0, None,
                                    OP.mult)
            t1 = sb.tile([1, J], F32, tag="t1")
            nc.vector.tensor_tensor(t1[:], gmm[:, J:2 * J], rden[:], OP.mult)
            nc.vector.tensor_scalar(coefr[:, J:2 * J], t1[:], 10.0, -0.5,
                                    OP.mult, OP.add)
            coef = sb.tile([128, 2 * J], F32, tag="coef")
            nc.gpsimd.partition_broadcast(coef[:], coefr[:])

            # ---- binning: binint = RNE(psS*s1 + b1) as int16 ----
            binf = sb.tile([128, NCH * J], F32, tag="binf")
            bf3 = binf[:].rearrange("p (c j) -> p c j", j=J)
            nc.vector.tensor_tensor(
                bf3,
                psS[:].rearrange("p (c j) -> p c j", j=2 * J)[:, :, 0:J],
                coef[:, None, 0:J].broadcast_to([128, NCH, J]), OP.mult)
            binint = sb.tile([128, NCH * J], I16, tag="binint")
            nc.vector.tensor_tensor(
                binint[:].rearrange("p (c j) -> p c j", j=J), bf3,
                coef[:, None, J:2 * J].broadcast_to([128, NCH, J]), OP.add)

            # ---- one-hot (7 bins on DVE, 3 on Pool; edge bins clamp) ----
            ohsb = sb.tile([128, NCH * J * NB], F16, tag="ohsb")
            oh4 = ohsb[:].rearrange("p (c j b) -> p c j b", j=J, b=NB)
            bi3 = binint[:].rearrange("p (c j) -> p c j", j=J)
            for b in range(NB):
                eng = nc.vector if b < 7 else nc.gpsimd
                op = OP.is_le if b == 0 else (OP.is_ge if b == NB - 1
                                              else OP.is_equal)
                eng.tensor_scalar(oh4[:, :, :, b], bi3, b, None, op)

            # ---- joint histograms: pair p -> its own PSUM bank tile ----
            psJp = [psJ_pool.tile([NB, NB], F32, tag="psJ", name=f"psJ{p}")
                    for p in range(NPAIR)]
            for c in range(NCH):
                base = c * J * NB
                for p in range(NPAIR):
                    nc.tensor.matmul(
                        psJp[p][:],
                        ohsb[:, base + NB * p:base + NB * (p + 1)],
                        ohsb[:, base + 40 + NB * p:base + 40 + NB * (p + 1)],
                        start=(c == 0), stop=(c == NCH - 1))
            jm = sb.tile([NB, NPAIR * NB], F32, tag="jm")
            nc.scalar.copy(jm[:, 0:NB], psJp[0][:])
            nc.vector.tensor_copy(jm[:, NB:2 * NB], psJp[1][:])
            nc.scalar.copy(jm[:, 2 * NB:3 * NB], psJp[2][:])
            nc.vector.tensor_copy(jm[:, 3 * NB:4 * NB], psJp[3][:])
            cbj = dram.tile([NPAIR, NB * NB], F32, tag="cbj")
            cbj2 = dram.tile([NPAIR, NB * NB], F32, tag="cbj2")
            nc.sync.dma_start(
                out=cbj[:].rearrange("p (a b) -> a p b", a=NB),
                in_=jm[:].rearrange("a (p b) -> a p b", b=NB))
            jnt = sb.tile([40, NB], F32, tag="jnt")
            if sim1:
                nc.sync.dma_start(
                    out=jnt[:], in_=cbj.rearrange("p (a b) -> (p a) b", a=NB))
            else:
                nc.gpsimd.collective_compute("AllReduce", OP.add,
                                             replica_groups=rg,
                                             ins=[cbj.opt()],
                                             outs=[cbj2.opt()])
                nc.sync.dma_start(
                    out=jnt[:], in_=cbj2.rearrange("p (a b) -> (p a) b", a=NB))

            # ---- MI, batched across pairs (pairs along partitions) ----
            # joint total is exactly T (each sample bins exactly once;
            # counts exact in fp32), and 1/T = 2^-15 is exact: bake it.
            TINV = float(1.0 / T)
            rowsum = sb.tile([40, 1], F32, tag="rowsum")
            nc.vector.tensor_reduce(rowsum[:], jnt[:], AX.X, OP.add)
            pscol = misc.tile([NPAIR, NB], F32, tag="m", name="pscol")
            nc.tensor.matmul(pscol[:], blkA, jnt[:], start=True, stop=True)
            px = sb.tile([40, 1], F32, tag="px")
            nc.vector.tensor_scalar(px[:], rowsum[:], TINV, None, OP.mult)
            py4 = sb.tile([NPAIR, NB], F32, tag="py4")
            nc.vector.tensor_scalar(py4[:], pscol[:], TINV, None, OP.mult)
            pspy = misc.tile([40, NB], F32, tag="m", name="pspy")
            nc.tensor.matmul(pspy[:], blkT, py4[:], start=True, stop=True)
            jn = sb.tile([40, NB], F32, tag="jn")
            nc.vector.tensor_scalar(jn[:], jnt[:], TINV, None, OP.mult)
            outer = sb.tile([40, NB], F32, tag="outer")
            nc.vector.tensor_scalar(outer[:], pspy[:], px[:], 1e-10, OP.mult,
                                    OP.add)
            rout = sb.tile([40, NB], F32, tag="rout")
            nc.vector.reciprocal(rout[:], outer[:])
            num = sb.tile([40, NB], F32, tag="num")
            nc.vector.scalar_tensor_tensor(num[:], jn[:], 1e-10, rout[:],
                                           OP.add, OP.mult)
            lg = sb.tile([40, NB], F32, tag="lg")
            nc.scalar.activation(lg[:], num[:], ACT.Ln)
            nc.vector.tensor_tensor(lg[:], jn[:], lg[:], OP.mult)
            ms = sb.tile([40, 1], F32, tag="ms")
            nc.vector.tensor_reduce(ms[:], lg[:], AX.X, OP.add)
            psmi = misc.tile([NPAIR, 1], F32, tag="m", name="psmi")
            nc.tensor.matmul(psmi[:], blkA, ms[:], start=True, stop=True)
            mirow4 = sb.tile([NPAIR, 1], F32, tag="mirow4")
            nc.vector.tensor_scalar(mirow4[:], psmi[:], 0.0, None, OP.max)
            psmT = misc.tile([1, NPAIR], F32, tag="m", name="psmT")
            nc.tensor.matmul(psmT[:], mirow4[:], id4, start=True, stop=True)
            nc.vector.tensor_copy(outrow[:, 5:9], psmT[:])
            nc.vector.tensor_reduce(outrow[:, 4:5], psmT[:], AX.X, OP.min)
            nc.vector.tensor_tensor(outrow[:, 0:1], outrow[:, 4:5], tanhd[:],
                                    OP.add)
            nc.sync.dma_start(out=out[:], in_=outrow[:])
            if debug:
                dbs = sb.tile([128, 16], F32, tag="dbs")
                nc.scalar.copy(dbs[:], psS[:, 0:16])
                nc.sync.dma_start(out=dbg_s[:], in_=dbs[:])
                nc.sync.dma_start(out=dbg_mm[:], in_=gmm[:])
                nc.sync.dma_start(out=dbg_bin[:], in_=binint[:, 0:16])
                nc.sync.dma_start(out=dbg_jnt[:], in_=jnt[:])
                nc.sync.dma_start(out=dbg_mi[:], in_=outrow[:])

    nc.compile()
    return nc


def mybir_reduce_max():
    import concourse.bass_isa as bass_isa
    return bass_isa.ReduceOp.max


def _get_nc(debug=False):
    key = ("ncd" if debug else "nc")
    if key not in _CACHE:
        _CACHE[key] = _build(debug)
    return _CACHE[key]


def kernel(state, state_memory, state_history, partitions, sample_idx,
           trace=False, debug=False):
    global LAST_RESULTS
    state = np.asarray(state, np.float32)
    state_memory = np.asarray(state_memory, np.float32)
    state_history = np.asarray(state_history, np.float32)
    partitions = np.asarray(partitions)
    sample_idx = np.asarray(sample_idx)

    pf = partitions.astype(np.float32)
    m8 = np.empty((D, 2 * J), np.float16)
    invc8 = np.empty((J,), np.float32)
    for p in range(NPAIR):
        m8[:, p] = pf[p]
        m8[:, NPAIR + p] = np.float32(1.0) - pf[p]
        invc8[p] = np.float32(1.0) / pf[p].sum(dtype=np.float32)
        invc8[NPAIR + p] = np.float32(1.0) / (np.float32(1.0) - pf[p]).sum(
            dtype=np.float32)
    m8[:, 0:J] = (m8[:, 0:J].astype(np.float32) * invc8[None, :]).astype(
        np.float16)
    m8[:, J:2 * J] = -m8[:, 0:J]
    cst1 = np.zeros((40, 4), np.float32)
    for p in range(NPAIR):
        cst1[NB * p:NB * (p + 1), p] = 1.0
    cst2 = np.zeros((4, 44), np.float32)
    for p in range(NPAIR):
        cst2[p, NB * p:NB * (p + 1)] = 1.0
        cst2[p, 40 + p] = 1.0

    memory = np.concatenate([state, state_memory[state.shape[0]:]], axis=0)
    memt = np.ascontiguousarray(memory.T).astype(np.float16)
    sampt = np.ascontiguousarray(memory[sample_idx].T).astype(np.float16)

    in_maps = []
    for c in range(N_CORES):
        htc = np.ascontiguousarray(
            state_history[c * TL:(c + 1) * TL, :].T).astype(np.float16)
        in_maps.append({"ht": htc, "m8": m8, "memt": memt, "sampt": sampt,
                        "cst1": cst1, "cst2": cst2})

    nc = _get_nc(debug)
    res = run_bass_kernel_spmd(nc, in_maps, list(range(N_CORES)),
                               trace=trace)
    LAST_RESULTS = res
    return np.asarray(res.results[0]["out"], np.float32)


# revision 21
# speedup vs baseline: 2.2367x; 1.0180x over previous
"""Trainium2 Bass kernel for nn_ConsciousnessMonitor (histogram_binning).

kernel(**inputs) takes FULL unsharded numpy inputs, returns the full (9,)
float32 output. Shards state_history along time across 8 NeuronCores.

Design: state_history streamed as fp16 (halves HBM traffic; empirically
max-rel-err from the cast is 6.9e-3 vs the 2e-2 gate). The masked-mean
matmul runs transposed (H chunk stationary, 8 mask columns moving) so the
projections land time-on-partitions in a single PSUM region [128, 32*8];
that removes the affine/transpose stage entirely. Min/max reductions split
across DVE and Pool, then a 64B AllReduce of (max,-min); binning is two
DVE ops + 10 one-hot compares (DVE/Pool split) + 32 packed 40x40 joint
matmuls; a 1.6KB AllReduce of the 4 joint histograms; the MI tail is
batched across all 4 pairs (pairs stacked along partitions).

Self-contained: shapes/sharding hardcoded; reads no sibling files.
"""
import numpy as np

import concourse.bacc as bacc
import concourse.tile as tile
import concourse.mybir as mybir
from concourse.bass_utils import run_bass_kernel_spmd

F32 = mybir.dt.float32
F16 = mybir.dt.float16
I16 = mybir.dt.int16
AX = mybir.AxisListType
OP = mybir.AluOpType
ACT = mybir.ActivationFunctionType

N_CORES = 8
T, D = 32768, 2048
TL = T // N_CORES          # 4096 time steps per core
NB = 10                    # histogram bins per axis
NPAIR = 4                  # partitions (mask pairs)
J = 2 * NPAIR              # 8 masked-sum series (x0..x3, y0..y3)
NDC = D // 128             # 16 contraction chunks
NCH = TL // 128            # 32 time chunks of 128 (partition-major)
MEM = 100
SN = 10

# DMA segments: (dk, t0, t1); last chunk split so its matmul tail is short
SEGS = [(dk, 0, TL) for dk in range(NDC - 1)]
SEGS += [(NDC - 1, 0, TL // 2), (NDC - 1, TL // 2, TL)]

_CACHE = {}
LAST_RESULTS = None


def _build(debug=False, variant="main"):
    sim1 = variant.startswith("sim1")
    nc = bacc.Bacc("TRN2", target_bir_lowering=False, debug=False,
                   num_devices=1 if sim1 else N_CORES)
    ht = nc.dram_tensor("ht", [D, TL], F16, kind="ExternalInput").ap()
    # m8/memt/sampt are host-pre-arranged to the SBUF [128, k*f] layout so
    # the DMAs use large contiguous descriptors
    m8 = nc.dram_tensor("m8", [128, NDC * 2 * J], F16,
                        kind="ExternalInput").ap()
    memt = nc.dram_tensor("memt", [128, NDC * MEM], F16,
                          kind="ExternalInput").ap()
    sampt = nc.dram_tensor("sampt", [128, NDC * SN], F16,
                           kind="ExternalInput").ap()
    # cst1 = blkones [40, 4] block indicator
    cst1 = nc.dram_tensor("cst1", [40, 4], F32, kind="ExternalInput").ap()
    # cst2: [4, 44] = blkT [4,40] | ident4 [4,4]
    cst2 = nc.dram_tensor("cst2", [4, 44], F32, kind="ExternalInput").ap()
    out = nc.dram_tensor("out", [9], F32, kind="ExternalOutput").ap()
    if debug:
        dbg_s = nc.dram_tensor("dbg_s", [128, 16], F32, kind="ExternalOutput").ap()
        dbg_mm = nc.dram_tensor("dbg_mm", [1, 16], F32, kind="ExternalOutput").ap()
        dbg_bin = nc.dram_tensor("dbg_bin", [128, 16], I16, kind="ExternalOutput").ap()
        dbg_jnt = nc.dram_tensor("dbg_jnt", [40, 10], F32, kind="ExternalOutput").ap()
        dbg_mi = nc.dram_tensor("dbg_mi", [1, 9], F32, kind="ExternalOutput").ap()

    rg = [list(range(N_CORES))]

    with tile.TileContext(nc) as tc:
        with tc.tile_pool(name="consts", bufs=1) as consts, \
             tc.tile_pool(name="sb", bufs=1) as sb, \
             tc.tile_pool(name="htp", bufs=4) as htp, \
             tc.tile_pool(name="psS", bufs=1, space="PSUM") as psS_pool, \
             tc.tile_pool(name="psJ", bufs=4, space="PSUM") as psJ_pool, \
             tc.tile_pool(name="misc", bufs=3, space="PSUM") as misc, \
             tc.tile_pool(name="dram", bufs=1, space="DRAM") as dram:

            # ---- mask matrix first: gates the stream matmuls ----
            m_sb = consts.tile([128, NDC * 2 * J], F16, tag="msb")
            nc.scalar.dma_start(out=m_sb[:], in_=m8[:])
            ones128 = consts.tile([128, 1], F32, tag="o128")
            nc.gpsimd.memset(ones128[:], 1.0)
            ones10 = consts.tile([SN, 1], F32, tag="o10")
            nc.gpsimd.memset(ones10[:], 1.0)
            ones1_10 = consts.tile([1, SN], F32, tag="o110")
            nc.vector.memset(ones1_10[:], 1.0)

            # ---- stream: psS[t128, (tc, j)] += ht_chunk.T @ m ----
            # moving operand = [masks | -masks]: psS[:, :, 8:16] = -S, so
            # one X-reduce max yields (max | -min) directly
            psS = psS_pool.tile([128, NCH * 2 * J], F32, tag="psS")
            for si, (dk, t0, t1) in enumerate(SEGS):
                htt = htp.tile([128, t1 - t0], F16, tag="htt", name="htt")
                nc.sync.dma_start(out=htt[:], in_=ht[dk * 128:(dk + 1) * 128,
                                                     t0:t1])
                for tcn in range(t0 // 128, t1 // 128):
                    # start=True zeroes the whole 2KB PSUM region, so only
                    # the very first matmul starts; only the last stops
                    nc.tensor.matmul(
                        psS[:, tcn * 2 * J:(tcn + 1) * 2 * J],
                        htt[:, tcn * 128 - t0:(tcn + 1) * 128 - t0],
                        m_sb[:, dk * 2 * J:(dk + 1) * 2 * J],
                        start=(si == 0 and tcn == 0),
                        stop=(si == len(SEGS) - 1 and tcn == NCH - 1))

            # ---- small inputs: issued late so their DMA slots fall after
            # the stream on the shared DMA engines; consumers have slack ----
            cst1_sb = consts.tile([40, 4], F32, tag="cst1")
            nc.scalar.dma_start(out=cst1_sb[:], in_=cst1[:])
            cst2_sb = consts.tile([4, 44], F32, tag="cst2")
            nc.scalar.dma_start(out=cst2_sb[:], in_=cst2[:])
            blkA = cst1_sb[:]               # [40,4] block indicator
            blkT = cst2_sb[:, 0:40]         # [4,40]
            id4 = cst2_sb[:, 40:44]         # [4,4]
            mem_sb = consts.tile([128, NDC * MEM], F16, tag="memsb")
            nc.scalar.dma_start(out=mem_sb[:], in_=memt[:])
            samp_sb = consts.tile([128, NDC * SN], F16, tag="sampsb")
            nc.scalar.dma_start(out=samp_sb[:], in_=sampt[:])

            # ---- (max | -min) in one reduce, partition-reduce, AllReduce --
            pboth = sb.tile([128, 2 * J], F32, tag="pboth")
            nc.vector.tensor_reduce(
                pboth[:], psS[:].rearrange("p (c g) -> p g c", g=2 * J),
                AX.X, OP.max)
            arb = sb.tile([128, 2 * J], F32, tag="arb")
            nc.gpsimd.partition_all_reduce(arb[:], pboth[:], 128,
                                           mybir_reduce_max())
            cbA = dram.tile([1, 2 * J], F32, tag="cba")
            cbB = dram.tile([1, 2 * J], F32, tag="cbb")
            nc.sync.dma_start(out=cbA[:], in_=arb[0:1, :])
            gmm = sb.tile([1, 2 * J], F32, tag="gmm")
            if sim1:
                nc.scalar.dma_start(out=gmm[:], in_=cbA[:])
            else:
                nc.gpsimd.collective_compute("AllReduce", OP.max,
                                             replica_groups=rg,
                                             ins=[cbA.opt()],
                                             outs=[cbB.opt()])
                nc.scalar.dma_start(out=gmm[:], in_=cbB[:])

            # ---- differentiation branch (fills the AllReduce wait) ----
            outrow = sb.tile([1, 9], F32, tag="outrow")
            psG = misc.tile([SN, SN], F32, tag="m", name="psG")
            for k in range(NDC):
                nc.tensor.matmul(psG[:], samp_sb[:, k * SN:(k + 1) * SN],
                                 samp_sb[:, k * SN:(k + 1) * SN],
                                 start=(k == 0), stop=(k == NDC - 1))
            sqs = sb.tile([128, NDC * SN], F32, tag="sqs")
            nc.vector.tensor_tensor(sqs[:], samp_sb[:], samp_sb[:], OP.mult)
            psrc = misc.tile([SN, 1], F32, tag="m", name="psrc")
            psrr = misc.tile([1, SN], F32, tag="m", name="psrr")
            for k in range(NDC):
                nc.tensor.matmul(psrc[:], sqs[:, k * SN:(k + 1) * SN],
                                 ones128[:], start=(k == 0),
                                 stop=(k == NDC - 1))
                nc.tensor.matmul(psrr[:], ones128[:],
                                 sqs[:, k * SN:(k + 1) * SN], start=(k == 0),
                                 stop=(k == NDC - 1))
            g_sb = sb.tile([SN, SN], F32, tag="gsb")
            nc.scalar.copy(g_sb[:], psG[:])
            rcol = sb.tile([SN, 1], F32, tag="rcol")
            nc.scalar.copy(rcol[:], psrc[:])
            rrow = sb.tile([1, SN], F32, tag="rrow")
            nc.scalar.copy(rrow[:], psrr[:])

            mem3 = mem_sb[:].rearrange("p (k f) -> p k f", f=MEM)
            mean16 = sb.tile([128, NDC], F32, tag="mean16")
            nc.vector.tensor_reduce(mean16[:], mem3, AX.X, OP.add)
            nc.vector.tensor_scalar(mean16[:], mean16[:], 1.0 / MEM, None,
                                    OP.mult)
            cent = sb.tile([128, NDC * MEM], F32, tag="cent")
            nc.vector.tensor_tensor(
                cent[:].rearrange("p (k f) -> p k f", f=MEM), mem3,
                mean16[:, :, None].broadcast_to([128, NDC, MEM]), OP.subtract)
            nc.vector.tensor_tensor(cent[:], cent[:], cent[:], OP.mult)
            var16 = sb.tile([128, NDC], F32, tag="var16")
            nc.vector.tensor_reduce(
                var16[:], cent[:].rearrange("p (k f) -> p k f", f=MEM),
                AX.X, OP.add)
            nc.vector.tensor_scalar(var16[:], var16[:], 1.0 / (MEM - 1), None,
                                    OP.mult)
            redv = sb.tile([128, 1], F32, tag="redv")
            nc.vector.tensor_reduce(redv[:], var16[:], AX.X, OP.add)
            v2 = sb.tile([128, NDC], F32, tag="v2")
            nc.vector.tensor_tensor(v2[:], var16[:], var16[:], OP.mult)
            redv2 = sb.tile([128, 1], F32, tag="redv2")
            nc.vector.tensor_reduce(redv2[:], v2[:], AX.X, OP.add)
            pstv = misc.tile([1, 1], F32, tag="m", name="pstv")
            nc.tensor.matmul(pstv[:], redv[:], ones128[:], start=True,
                             stop=True)
            tv_sb = outrow[:, 3:4]
            nc.scalar.copy(tv_sb, pstv[:])
            pss2 = misc.tile([1, 1], F32, tag="m", name="pss2")
            nc.tensor.matmul(pss2[:], redv2[:], ones128[:], start=True,
                             stop=True)
            s2_sb = sb.tile([1, 1], F32, tag="s2sb")
            nc.scalar.copy(s2_sb[:], pss2[:])

            tvsq = sb.tile([1, 1], F32, tag="tvsq")
            nc.vector.tensor_tensor(tvsq[:], tv_sb, tv_sb, OP.mult)
            dden = sb.tile([1, 1], F32, tag="dden")
            nc.vector.scalar_tensor_tensor(dden[:], tvsq[:], 1e-6, s2_sb[:],
                                           OP.mult, OP.add)
            rdden = sb.tile([1, 1], F32, tag="rdden")
            nc.vector.reciprocal(rdden[:], dden[:])
            nc.vector.tensor_tensor(outrow[:, 2:3], tvsq[:], rdden[:],
                                    OP.mult)

            # cdist tail: d2 = r_i + r_j - 2G
            rB = misc.tile([SN, SN], F32, tag="m", name="rB")
            nc.tensor.matmul(rB[:], ones1_10[:], rrow[:], start=True,
                             stop=True)
            d2 = sb.tile([SN, SN], F32, tag="d2")
            nc.vector.scalar_tensor_tensor(d2[:], g_sb[:], -2.0, rB[:],
                                           OP.mult, OP.add)
            nc.vector.tensor_scalar(d2[:], d2[:], rcol[:], 0.0, OP.add,
                                    OP.max)
            dst = sb.tile([SN, SN], F32, tag="dst")
            nc.scalar.activation(dst[:], d2[:], ACT.Sqrt)
            dsum = sb.tile([SN, 1], F32, tag="dsum")
            nc.vector.tensor_reduce(dsum[:], dst[:], AX.X, OP.add)
            psD = misc.tile([1, 1], F32, tag="m", name="psD")
            nc.tensor.matmul(psD[:], dsum[:], ones10[:], start=True, stop=True)
            avg_sb = sb.tile([1, 1], F32, tag="avgsb")
            nc.vector.tensor_scalar(avg_sb[:], psD[:],
                                    float(1.0 / (SN * (SN - 1) + 1e-6)), None,
                                    OP.mult)
            sqtv = sb.tile([1, 1], F32, tag="sqtv")
            nc.scalar.activation(sqtv[:], tv_sb, ACT.Sqrt)
            diff_sb = outrow[:, 1:2]
            nc.vector.tensor_tensor(diff_sb, sqtv[:], avg_sb[:], OP.mult)
            tanhd = sb.tile([1, 1], F32, tag="tanhd")
            nc.scalar.activation(tanhd[:], diff_sb, ACT.Tanh)
            # preload the Ln table during collective slack so the MI-tail
            # Ln doesn't pay the 1.3us table switch (copies stay in-set)
            lnwarm = sb.tile([1, 1], F32, tag="lnwarm")
            nc.scalar.activation(lnwarm[:], tanhd[:], ACT.Ln)

            # ---- bin coefficients from global (vmax | -vmin); masks are
            # pre-scaled by 1/count so psS holds mean-scale values ----
            # s = 10/(vmax-vmin+1e-6); s1 = s; b1 = -vmin*s - 0.5
            den = sb.tile([1, J], F32, tag="den")
            nc.vector.tensor_tensor(den[:], gmm[:, 0:J], gmm[:, J:2 * J],
                                    OP.add)
            nc.vector.tensor_scalar(den[:], den[:], 1e-6, None, OP.add)
            rden = sb.tile([1, J], F32, tag="rden")
            nc.vector.reciprocal(rden[:], den[:])
            coefr = sb.tile([1, 2 * J], F32, tag="coefr")
            nc.vector.tensor_scalar(coefr[:, 0:J], rden[:], 10.0, None,
                                    OP.mult)
            t1 = sb.tile([1, J], F32, tag="t1")
            nc.vector.tensor_tensor(t1[:], gmm[:, J:2 * J], rden[:], OP.mult)
            nc.vector.tensor_scalar(coefr[:, J:2 * J], t1[:], 10.0, -0.5,
                                    OP.mult, OP.add)
            coef = sb.tile([128, 2 * J], F32, tag="coef")
            nc.gpsimd.partition_broadcast(coef[:], coefr[:])

            # ---- binning: binint = RNE(psS*s1 + b1) as int16 ----
            binf = sb.tile([128, NCH * J], F32, tag="binf")
            bf3 = binf[:].rearrange("p (c j) -> p c j", j=J)
            nc.vector.tensor_tensor(
                bf3,
                psS[:].rearrange("p (c j) -> p c j", j=2 * J)[:, :, 0:J],
                coef[:, None, 0:J].broadcast_to([128, NCH, J]), OP.mult)
            binint = sb.tile([128, NCH * J], I16, tag="binint")
            nc.vector.tensor_tensor(
                binint[:].rearrange("p (c j) -> p c j", j=J), bf3,
                coef[:, None, J:2 * J].broadcast_to([128, NCH, J]), OP.add)

            # ---- one-hot (7 bins on DVE, 3 on Pool; edge bins clamp) ----
            ohsb = sb.tile([128, NCH * J * NB], F16, tag="ohsb")
            oh4 = ohsb[:].rearrange("p (c j b) -> p c j b", j=J, b=NB)
            bi3 = binint[:].rearrange("p (c j) -> p c j", j=J)
            for b in range(NB):
                eng = nc.vector if b < 7 else nc.gpsimd
                op = OP.is_le if b == 0 else (OP.is_ge if b == NB - 1
                                              else OP.is_equal)
                eng.tensor_scalar(oh4[:, :, :, b], bi3, b, None, op)

            # ---- joint histograms: pair p -> its own PSUM bank tile ----
            psJp = [psJ_pool.tile([NB, NB], F32, tag="psJ", name=f"psJ{p}")
                    for p in range(NPAIR)]
            for c in range(NCH):
                base = c * J * NB
                for p in range(NPAIR):
                    nc.tensor.matmul(
                        psJp[p][:],
                        ohsb[:, base + NB * p:base + NB * (p + 1)],
                        ohsb[:, base + 40 + NB * p:base + 40 + NB * (p + 1)],
                        start=(c == 0), stop=(c == NCH - 1))
            jm = sb.tile([NB, NPAIR * NB], F32, tag="jm")
            nc.scalar.copy(jm[:, 0:NB], psJp[0][:])
            nc.vector.tensor_copy(jm[:, NB:2 * NB], psJp[1][:])
            nc.scalar.copy(jm[:, 2 * NB:3 * NB], psJp[2][:])
            nc.vector.tensor_copy(jm[:, 3 * NB:4 * NB], psJp[3][:])
            cbj = dram.tile([NPAIR, NB * NB], F32, tag="cbj")
            cbj2 = dram.tile([NPAIR, NB * NB], F32, tag="cbj2")
            nc.sync.dma_start(
                out=cbj[:].rearrange("p (a b) -> a p b", a=NB),
                in_=jm[:].rearrange("a (p b) -> a p b", b=NB))
            jnt = sb.tile([40, NB], F32, tag="jnt")
            if sim1:
                nc.sync.dma_start(
                    out=jnt[:], in_=cbj.rearrange("p (a b) -> (p a) b", a=NB))
            else:
                nc.gpsimd.collective_compute("AllReduce", OP.add,
                                             replica_groups=rg,
                                             ins=[cbj.opt()],
                                             outs=[cbj2.opt()])
                nc.sync.dma_start(
                    out=jnt[:], in_=cbj2.rearrange("p (a b) -> (p a) b", a=NB))

            # ---- MI, batched across pairs (pairs along partitions) ----
            # joint total is exactly T (each sample bins exactly once;
            # counts exact in fp32), and 1/T = 2^-15 is exact: bake it.
            TINV = float(1.0 / T)
            rowsum = sb.tile([40, 1], F32, tag="rowsum")
            nc.vector.tensor_reduce(rowsum[:], jnt[:], AX.X, OP.add)
            pscol = misc.tile([NPAIR, NB], F32, tag="m", name="pscol")
            nc.tensor.matmul(pscol[:], blkA, jnt[:], start=True, stop=True)
            px = sb.tile([40, 1], F32, tag="px")
            nc.vector.tensor_scalar(px[:], rowsum[:], TINV, None, OP.mult)
            py4 = sb.tile([NPAIR, NB], F32, tag="py4")
            nc.vector.tensor_scalar(py4[:], pscol[:], TINV, None, OP.mult)
            pspy = misc.tile([40, NB], F32, tag="m", name="pspy")
            nc.tensor.matmul(pspy[:], blkT, py4[:], start=True, stop=True)
            jn = sb.tile([40, NB], F32, tag="jn")
            nc.vector.tensor_scalar(jn[:], jnt[:], TINV, None, OP.mult)
            outer = sb.tile([40, NB], F32, tag="outer")
            nc.vector.tensor_scalar(outer[:], pspy[:], px[:], 1e-10, OP.mult,
                                    OP.add)
            rout = sb.tile([40, NB], F32, tag="rout")
            nc.vector.reciprocal(rout[:], outer[:])
            num = sb.tile([40, NB], F32, tag="num")
            nc.vector.scalar_tensor_tensor(num[:], jn[:], 1e-10, rout[:],
                                           OP.add, OP.mult)
            lg = sb.tile([40, NB], F32, tag="lg")
            nc.scalar.activation(lg[:], num[:], ACT.Ln)
            nc.vector.tensor_tensor(lg[:], jn[:], lg[:], OP.mult)
            ms = sb.tile([40, 1], F32, tag="ms")
            nc.vector.tensor_reduce(ms[:], lg[:], AX.X, OP.add)
            # per-pair MI sums as a row in one matmul: [1,4] = ms^T @ blkA
            psmT = misc.tile([1, NPAIR], F32, tag="m", name="psmT")
            nc.tensor.matmul(psmT[:], ms[:], blkA, start=True, stop=True)
            nc.vector.tensor_scalar(outrow[:, 5:9], psmT[:], 0.0, None,
                                    OP.max)
            nc.vector.tensor_reduce(outrow[:, 4:5], outrow[:, 5:9], AX.X,
                                    OP.min)
            nc.vector.tensor_tensor(outrow[:, 0:1], outrow[:, 4:5], tanhd[:],
                                    OP.add)
            nc.sync.dma_start(out=out[:], in_=outrow[:])
            if debug:
                dbs = sb.tile([128, 16], F32, tag="dbs")
                nc.scalar.copy(dbs[:], psS[:, 0:16])
                nc.sync.dma_start(out=dbg_s[:], in_=dbs[:])
                nc.sync.dma_start(out=dbg_mm[:], in_=gmm[:])
                nc.sync.dma_start(out=dbg_bin[:], in_=binint[:, 0:16])
                nc.sync.dma_start(out=dbg_jnt[:], in_=jnt[:])
                nc.sync.dma_start(out=dbg_mi[:], in_=outrow[:])

    nc.compile()
    return nc


def mybir_reduce_max():
    import concourse.bass_isa as bass_isa
    return bass_isa.ReduceOp.max


def _get_nc(debug=False):
    key = ("ncd" if debug else "nc")
    if key not in _CACHE:
        _CACHE[key] = _build(debug)
    return _CACHE[key]


def kernel(state, state_memory, state_history, partitions, sample_idx,
           trace=False, debug=False):
    global LAST_RESULTS
    state = np.asarray(state, np.float32)
    state_memory = np.asarray(state_memory, np.float32)
    state_history = np.asarray(state_history, np.float32)
    partitions = np.asarray(partitions)
    sample_idx = np.asarray(sample_idx)

    pf = partitions.astype(np.float32)
    m8 = np.empty((D, 2 * J), np.float16)
    invc8 = np.empty((J,), np.float32)
    for p in range(NPAIR):
        m8[:, p] = pf[p]
        m8[:, NPAIR + p] = np.float32(1.0) - pf[p]
        invc8[p] = np.float32(1.0) / pf[p].sum(dtype=np.float32)
        invc8[NPAIR + p] = np.float32(1.0) / (np.float32(1.0) - pf[p]).sum(
            dtype=np.float32)
    m8[:, 0:J] = (m8[:, 0:J].astype(np.float32) * invc8[None, :]).astype(
        np.float16)
    m8[:, J:2 * J] = -m8[:, 0:J]
    cst1 = np.zeros((40, 4), np.float32)
    for p in range(NPAIR):
        cst1[NB * p:NB * (p + 1), p] = 1.0
    cst2 = np.zeros((4, 44), np.float32)
    for p in range(NPAIR):
        cst2[p, NB * p:NB * (p + 1)] = 1.0
        cst2[p, 40 + p] = 1.0

    def to_pk(a):
        # [D, f] -> [128, NDC*f] with row p holding chunks k at d = k*128+p
        f = a.shape[1]
        return np.ascontiguousarray(
            a.reshape(NDC, 128, f).transpose(1, 0, 2).reshape(128, NDC * f))

    memory = np.concatenate([state, state_memory[state.shape[0]:]], axis=0)
    memt = to_pk(memory.T.astype(np.float16))
    sampt = to_pk(memory[sample_idx].T.astype(np.float16))
    m8 = to_pk(m8)

    in_maps = []
    for c in range(N_CORES):
        htc = np.ascontiguousarray(
            state_history[c * TL:(c + 1) * TL, :].T).astype(np.float16)
        in_maps.append({"ht": htc, "m8": m8, "memt": memt, "sampt": sampt,
                        "cst1": cst1, "cst2": cst2})

    nc = _get_nc(debug)
    res = run_bass_kernel_spmd(nc, in_maps, list(range(N_CORES)),
                               trace=trace)
    LAST_RESULTS = res
    return np.asarray(res.results[0]["out"], np.float32)
